# revision 64
# baseline (speedup 1.0000x reference)
"""DGLJTNNDecoder kernel for 8x Trainium2 NeuronCores (Bass/Tile), v3.

Tree-GRU decoder over B=512 chain-trees (N=48 nodes), T=94 DFS steps,
followed by two MLP heads producing (q_loss, p_loss, q_acc, p_acc).
Data-parallel over trees, 64 trees per core; host combines partials.

v3 highlights:
  - sigmoids as scaled tanh (sigma(x) = (1+tanh(x/2))/2, halving folded
    into host-prescaled Wz/bz/Wr/Ur/br) so everything up to the tail
    runs out of the exp_and_others ACT table set (tanh/exp/relu/
    identity/abs); a single table switch at the tail covers the Ln ops.
  - the fwd and bwd DFS chains share one interleaved history tile
    hist[128, kt, slot, chain, 64]: slot t holds exactly the two state
    vectors iteration t consumes, so every state matmul runs at N=128
    with one LDWEIGHTS per weight tile, and all scan elementwise/tanh
    ops are fused across chains (FD=256).
  - A_z and A_r live interleaved in one arena so a single N=256
    identity-matmul per chain prefills the z+r PSUM bank (biases are
    folded into the projections in phase B).
  - h_v is accumulated in node order: hbw[n] = m_bwd[slot 47-n] +
    m_fwd[slot n]  (also fixes v1's off-by-one).
  - gathers/weight DMAs fan out over four trigger queues and the
    embedding transposes chase them, so the prefix is short; head
    matmuls (phase-B remainder, p0/p1/q1/q2/p2) are pumped between
    scan iterations with psum evacuations deferred one iteration.
"""

import sys

if "/opt/trn_rl_repo" not in sys.path:
    sys.path.insert(0, "/opt/trn_rl_repo")

import numpy as np

B, N, H, L, V = 512, 48, 256, 64, 800
NC = 8
BC = B // NC            # 64 trees per core
NF = N - 1              # 47 forward steps (= backward steps)
T = 2 * NF              # 94
NODES = N * BC          # 3072 gathered node rows per core
QBLK = NF + 1           # 48 q-head blocks
PBLK = T + 1            # 95 p-head blocks
PROWS = PBLK * BC       # 6080
PPAD = 48 * 128         # 6144

BF16 = True

_CACHE = {}


def _build(wob_nonzero: bool):
    import concourse.bass as bass
    import concourse.tile as tile
    from concourse import bacc, mybir
    from concourse.masks import make_identity

    f32 = mybir.dt.float32
    i32 = mybir.dt.int32
    wdt = mybir.dt.bfloat16 if BF16 else f32
    AF = mybir.ActivationFunctionType
    ALU = mybir.AluOpType
    AX = mybir.AxisListType

    nc = bacc.Bacc()

    def din(name, shape, dtype=f32):
        return nc.declare_dram_parameter(name, list(shape), dtype, isOutput=False)

    gidx = din("gidx", [24, 128], i32)
    tvt = din("tvt", [L, 8 * BC], wdt)
    qtgt = din("qtgt", [128, 24])
    ptgt = din("ptgt", [128, 48])
    emb = din("emb", [V, H], wdt)
    WzT = din("WzT", [H, H], wdt); WzB = din("WzB", [H, H], wdt)
    WhT = din("WhT", [H, H], wdt); WhB = din("WhB", [H, H], wdt)
    Wr = din("Wr", [H, H], wdt); Ur = din("Ur", [H, H], wdt)
    UwX = din("UwX", [H, H], wdt); UwH = din("UwH", [H, H], wdt)
    UwL = din("UwL", [L, H], wdt)
    WwH = din("WwH", [H, H], wdt); WwL = din("WwL", [L, H], wdt)
    Wo = din("Wo", [H, V], wdt); Us = din("Us", [H, 1], wdt)
    bz2 = din("bz2", [128, 2]); bh2 = din("bh2", [128, 2]); br2 = din("br2", [128, 2])
    ub2 = din("ub2", [128, 2]); wb2 = din("wb2", [128, 2])
    usb = din("usb", [128, 1])
    wob = din("wob", [1, V]) if wob_nonzero else None
    outp = nc.declare_dram_parameter("outp", [128, 8], f32, isOutput=True)

    def rearr2(ap):
        return ap.rearrange("(k p) m -> p k m", p=128)

    with tile.TileContext(nc) as tc:
        with (
            tc.tile_pool(name="persist", bufs=1) as pp,
            tc.tile_pool(name="small", bufs=1) as sp,
        ):
            # --- DMA triggers fan out over queues (gpsimd reserved for
            # the indirect gathers, which only it can issue) --------------
            dma_queues = [nc.sync, nc.scalar]
            dq_i = [0]

            def dma_next(out, in_):
                eng = dma_queues[dq_i[0] % len(dma_queues)]
                dq_i[0] += 1
                eng.dma_start(out=out, in_=in_)

            idx_s = pp.tile([128, 24], i32, tag="idx")
            nc.sync.dma_start(out=idx_s, in_=gidx[:].rearrange("c p -> p c"))

            def loadw(dram, shape, tag, dt=wdt, re2=True):
                t = pp.tile(shape, dt, tag=tag)
                dma_next(t, rearr2(dram[:]) if re2 else dram[:])
                return t

            wzb_s = loadw(WzB, [128, 2, H], "wzb")
            whb_s = loadw(WhB, [128, 2, H], "whb")
            ur_s = loadw(Ur, [128, 2, H], "ur")
            wzt_s = loadw(WzT, [128, 2, H], "wzt")
            wht_s = loadw(WhT, [128, 2, H], "wht")
            wr_s = loadw(Wr, [128, 2, H], "wr")
            uwx_s = loadw(UwX, [128, 2, H], "uwx")
            uwh_s = loadw(UwH, [128, 2, H], "uwh")
            wwh_s = loadw(WwH, [128, 2, H], "wwh")
            wo_s = loadw(Wo, [128, 2, V], "wo")
            us_s = loadw(Us, [128, 2, 1], "us")
            uwl_s = loadw(UwL, [L, H], "uwl", re2=False)
            wwl_s = loadw(WwL, [L, H], "wwl", re2=False)
            bz_s = loadw(bz2, [128, 2], "bz", dt=f32, re2=False)
            bh_s = loadw(bh2, [128, 2], "bh", dt=f32, re2=False)
            br_s = loadw(br2, [128, 2], "br", dt=f32, re2=False)
            ub_s = loadw(ub2, [128, 2], "ub", dt=f32, re2=False)
            wb_s = loadw(wb2, [128, 2], "wb", dt=f32, re2=False)
            usb_s = loadw(usb, [128, 1], "usb", dt=f32, re2=False)
            qtgt_s = loadw(qtgt, [128, 24], "qtgt", dt=f32, re2=False)
            ptgt_s = loadw(ptgt, [128, 48], "ptgt", dt=f32, re2=False)
            wob_s = loadw(wob, [1, V], "wob", dt=f32, re2=False) if wob_nonzero else None

            tvrep = pp.tile([L, 8, BC], wdt, tag="tvrep")
            dma_next(tvrep, tvt[:].rearrange("l (r b) -> l r b", b=BC))

            ident = pp.tile([128, 128], wdt, tag="ident")
            make_identity(nc, ident)

            iota_f = pp.tile([128, V], f32, tag="iotaf")
            iota_i = pp.tile([128, V], i32, tag="iotai")
            nc.gpsimd.iota(iota_i, pattern=[[1, V]], base=0, channel_multiplier=0)
            nc.vector.tensor_copy(iota_f, iota_i)

            # persistent tensors
            xt = pp.tile([128, 2, NODES], wdt, tag="xt")
            hist = pp.tile([128, 2, QBLK, 2, BC], wdt, tag="hist")  # [kt,slot,ch,b]
            hbw = pp.tile([128, 2, QBLK, BC], wdt, tag="hbw")       # node order
            azr = pp.tile([128, 2, 2, NODES], wdt, tag="azr")       # [gate(z,r),mt,*]
            ah = pp.tile([128, 2, NODES], wdt, tag="ah")
            p0a = pp.tile([128, 2, NODES], wdt, tag="p0a")
            p1a = pp.tile([128, 2, NODES], wdt, tag="p1a")
            q1a = pp.tile([128, 2, NODES], wdt, tag="q1a")
            scr = pp.tile([128, 1024], f32, tag="scr")
            scr2 = pp.tile([128, 1024], f32, tag="scr2")

            nc.vector.memset(hist[:, :, 0, :, :], 0.0)
            nc.vector.memset(p1a[:, :, NF * BC:], 0.0)

            outp_s = sp.tile([128, 8], f32, tag="outp")
            nc.vector.memset(outp_s, 0.0)
            sume_acc = sp.tile([128, 48], f32, tag="sume")
            qt_acc = sp.tile([128, 48], f32, tag="qta")
            rmax_acc = sp.tile([128, 48], f32, tag="rmx")

            hbw_f = hbw.rearrange("p k s b -> p k (s b)")
            xt_v = xt.rearrange("p k (c f) -> p k c f", f=128)

            # mfq-equivalent strided views of hist (fwd chain, ch=0):
            def mfq_cols(kt, s0, ns):
                return hist[:, kt, s0 : s0 + ns, 0, :]

            # ================= scan-era pools ==========================
            # PSUM (8 banks): zr x2 + h x1 + head-pipe x5.
            with (
                tc.tile_pool(name="zrp", bufs=2, space="PSUM") as zr_p,
                tc.tile_pool(name="hp", bufs=1, space="PSUM") as h_p,
                tc.tile_pool(name="hps", bufs=5, space="PSUM") as hps_p,
                tc.tile_pool(name="st", bufs=4) as st,
            ):
                # --- Phase A: gathers land kt-split in xg_all; the XBAR
                # DMA transpose moves them into xt on the idle DMA engines
                # (no PE transposes, no psum evacs).  Chunk order matches
                # first use: bwd chain needs chunk 5 at iter 0.
                xg_all = pp.tile([128, 24, H], wdt, tag="xga")
                gather_order = [20, 21, 22, 23, 0, 1, 2, 3,
                                16, 17, 18, 19, 4, 5, 6, 7,
                                12, 13, 14, 15, 8, 9, 10, 11]
                for c in gather_order:
                    nc.gpsimd.indirect_dma_start(
                        out=xg_all[:, c, :], out_offset=None, in_=emb[:],
                        in_offset=bass.IndirectOffsetOnAxis(
                            ap=idx_s[:, c : c + 1], axis=0))
                xt_g = xt.rearrange("p k (g n) -> p k g n", n=128)

                def chunk_transpose(ch):
                    # xt[:, kt, g*128:(g+1)*128] = (gathered group g's
                    # feature half kt)^T, via plain 2D XBAR DMA transposes
                    # (the production SBUF->SBUF pattern).
                    for g in range(4 * ch, 4 * ch + 4):
                        for kt in range(2):
                            nc.sync.dma_start_transpose(
                                out=xt_g[:, kt, g, :],
                                in_=xg_all[:, g, kt * 128 : (kt + 1) * 128])

                # warm the PE clock (HAM) while the first gathers land:
                # harmless matmuls on the identity into a scratch bank.
                warm = hps_p.tile([128, 512], f32, tag="hps")
                for i in range(24):
                    nc.tensor.matmul(
                        warm[:, :128], ident, ident,
                        start=(i == 0), stop=(i == 23))

                for ch in (5, 0, 4, 1, 3, 2):
                    chunk_transpose(ch)

                # --- evac engine alternation (2:1 toward ACT; DVE is the
                # hotter engine during the era) --------------------------
                eng_flip = [0]

                def _evac_on_act():
                    eng_flip[0] = (eng_flip[0] + 1) % 3
                    return eng_flip[0] != 0

                def evac_relu(dst, ps, bias_ap):
                    if _evac_on_act():
                        nc.scalar.activation(dst, ps, AF.Relu, bias=bias_ap)
                    else:
                        nc.vector.tensor_scalar(
                            out=dst, in0=ps, scalar1=bias_ap, scalar2=0.0,
                            op0=ALU.add, op1=ALU.max)

                def evac_ident(dst, ps, bias_ap):
                    if _evac_on_act():
                        nc.scalar.activation(dst, ps, AF.Identity, bias=bias_ap)
                    else:
                        nc.vector.tensor_scalar(
                            out=dst, in0=ps, scalar1=bias_ap, scalar2=None,
                            op0=ALU.add)

                # --- units ---------------------------------------------
                def b_unit(mat, ch, mt):
                    w_s, b_s = {
                        "z": (wzt_s, bz_s), "h": (wht_s, bh_s),
                        "r": (wr_s, br_s)}[mat]
                    dst = {"z": azr[:, 0], "h": ah, "r": azr[:, 1]}[mat]
                    msl = slice(mt * 128, (mt + 1) * 128)
                    csl = slice(ch * 512, (ch + 1) * 512)
                    ps = hps_p.tile([128, 512], f32, tag="hps")
                    for kt in range(2):
                        nc.tensor.matmul(
                            ps, w_s[:, kt, msl], xt[:, kt, csl],
                            start=(kt == 0), stop=(kt == 1))
                    return lambda: evac_ident(dst[:, mt, csl], ps, b_s[:, mt:mt+1])

                def p0_unit(ch, mt):
                    msl = slice(mt * 128, (mt + 1) * 128)
                    csl = slice(ch * 512, (ch + 1) * 512)
                    ps = hps_p.tile([128, 512], f32, tag="hps")
                    for kt in range(2):
                        nc.tensor.matmul(
                            ps, uwx_s[:, kt, msl], xt[:, kt, csl],
                            start=(kt == 0), stop=False)
                    for kt in range(2):
                        nc.tensor.matmul(
                            ps, uwh_s[:, kt, msl], mfq_cols(kt, 8 * ch, 8),
                            start=False, stop=False)
                    nc.tensor.matmul(
                        ps, uwl_s[:, msl], tvrep[:, :8, :],
                        start=False, stop=True)
                    return lambda: evac_relu(p0a[:, mt, csl], ps, ub_s[:, mt:mt+1])

                def p1_unit(u, mt):
                    n0 = 4 * u
                    nn = min(4, NF - n0)
                    cw = nn * BC
                    msl = slice(mt * 128, (mt + 1) * 128)
                    csl = slice(n0 * BC, n0 * BC + cw)
                    ps = hps_p.tile([128, 512], f32, tag="hps")
                    psv = ps[:, :cw]
                    for kt in range(2):
                        nc.tensor.matmul(
                            psv, uwx_s[:, kt, msl], xt[:, kt, csl],
                            start=(kt == 0), stop=False)
                    for kt in range(2):
                        nc.tensor.matmul(
                            psv, uwh_s[:, kt, msl], hbw_f[:, kt, csl],
                            start=False, stop=False)
                    nc.tensor.matmul(
                        psv, uwl_s[:, msl], tvrep[:, :nn, :],
                        start=False, stop=True)
                    return lambda: evac_relu(p1a[:, mt, csl], psv, ub_s[:, mt:mt+1])

                def q1_unit(ch, mt):
                    msl = slice(mt * 128, (mt + 1) * 128)
                    csl = slice(ch * 512, (ch + 1) * 512)
                    ps = hps_p.tile([128, 512], f32, tag="hps")
                    for kt in range(2):
                        nc.tensor.matmul(
                            ps, wwh_s[:, kt, msl], mfq_cols(kt, 8 * ch, 8),
                            start=(kt == 0), stop=False)
                    nc.tensor.matmul(
                        ps, wwl_s[:, msl], tvrep[:, :8, :],
                        start=False, stop=True)
                    return lambda: evac_relu(q1a[:, mt, csl], ps, wb_s[:, mt:mt+1])

                def q2_unit(j, half):
                    # vocab half of one logits row-tile through the shared
                    # head-pipe pool; softmax partials (sume/qt/rmax) are
                    # accumulated per half and combined in three tail ops.
                    n0, nn = (0, 512) if half == 0 else (512, V - 512)
                    col = half * 24 + j
                    psq = hps_p.tile([128, 512], f32, tag="hps")
                    pv = psq[:, :nn]
                    jsl = slice(j * 128, (j + 1) * 128)
                    for kt in range(2):
                        nc.tensor.matmul(
                            pv, q1a[:, kt, jsl], wo_s[:, kt, n0 : n0 + nn],
                            start=(kt == 0), stop=(kt == 1))
                    if wob_nonzero:
                        wv = wob_s[:]
                        wb_b = bass.AP(
                            tensor=wv.tensor, offset=wv.offset + n0,
                            ap=[[0, 128], [1, nn]])
                        nc.vector.tensor_add(pv, pv, wb_b)

                    def softmax():
                        nc.scalar.activation(
                            scr[:, :nn], pv, AF.Exp,
                            accum_out=sume_acc[:, col : col + 1])
                        nc.vector.scalar_tensor_tensor(
                            out=scr2[:, :nn], in0=iota_f[:, n0 : n0 + nn],
                            scalar=qtgt_s[:, j : j + 1],
                            in1=pv, op0=ALU.is_equal, op1=ALU.mult,
                            accum_out=qt_acc[:, col : col + 1])
                        nc.vector.reduce_max(
                            rmax_acc[:, col : col + 1], pv, axis=AX.X)
                    return softmax

                # p2 sub-units: a few p1-row tiles reduced against Us as
                # soon as their relu arena columns are complete, straight
                # into the matching p_sb slice (no persistent psum).
                p_sb = sp.tile([128, 48], f32, tag="psb")

                def p2_unit(src, j0, nj, col0):
                    ps = hps_p.tile([128, 512], f32, tag="hps")
                    for j in range(j0, j0 + nj):
                        for kt in range(2):
                            nc.tensor.matmul(
                                ps[:, j - j0 : j - j0 + 1],
                                src[:, kt, j * 128 : (j + 1) * 128],
                                us_s[:, kt, :],
                                start=(kt == 0), stop=(kt == 1))
                    return lambda: nc.scalar.activation(
                        p_sb[:, col0 : col0 + nj], ps[:, :nj],
                        AF.Identity, bias=usb_s[:, 0:1])

                # --- pump scheduler ------------------------------------
                pending = []
                seq_ctr = [0]

                def enq(ready, kind, fn, deadline=10**9):
                    pending.append([ready, seq_ctr[0], kind, deadline, fn])
                    seq_ctr[0] += 1

                evacs_next = []
                q1_emitted = {}
                p0_emitted = {}
                p1_emitted = {}
                cur_t = [0]

                def flush_evacs():
                    for ev in evacs_next:
                        ev()
                    evacs_next.clear()

                def pump(t, nh=3):
                    flush_evacs()
                    budget = {"hps": nh}
                    pending.sort(key=lambda u: (u[0], u[1]))
                    for u in list(pending):
                        ready, _, kind, deadline, fn = u
                        if ready > t or budget[kind] == 0:
                            continue
                        assert t <= deadline, f"unit past deadline at iter {t}"
                        budget[kind] -= 1
                        pending.remove(u)
                        evacs_next.append(fn())

                # phase-B chunks 5,0 then 4,1 in the prefix (the scan
                # reads them by iters 0 and 8); software-pipelined evacs
                # (hps bufs=2 -> <=2 in flight).
                def b_prefix(chunks):
                    prev_ev = None
                    for ch in chunks:
                        for mat in ("z", "r", "h"):
                            for mt in range(2):
                                ev = b_unit(mat, ch, mt)
                                if prev_ev is not None:
                                    prev_ev()
                                prev_ev = ev
                    prev_ev()

                b_prefix((5, 0))
                b_prefix((4, 1))
                # remaining B chunks (first read at scan iter 16 ->
                # emission deadline 14); their xt transposes were issued
                # in the prefix, so they are ready immediately.
                for ch, rdy, dl in ((3, 0, 16), (2, 2, 16)):
                    for mat in ("z", "r", "h"):
                        for mt in range(2):
                            enq(rdy, "hps",
                                lambda m=mat, c=ch, k=mt: b_unit(m, c, k),
                                deadline=dl - 2)

                def _after_p0(c, k):
                    ev = p0_unit(c, k)
                    def done():
                        ev()
                        p0_emitted[c] = p0_emitted.get(c, 0) + 1
                        if p0_emitted[c] == 2:
                            # p0a chunk c complete -> p-row tiles 4c..4c+3
                            enq(cur_t[0] + 1, "hps",
                                lambda cc=c: p2_unit(p0a, 4 * cc, 4, 4 * cc))
                    return done

                def _after_p1(u, k):
                    ev = p1_unit(u, k)
                    def done():
                        ev()
                        p1_emitted[u] = p1_emitted.get(u, 0) + 1
                        if p1_emitted[u] == 2:
                            # p1a nodes 4u..4u+3 complete -> tiles 2u,2u+1
                            enq(cur_t[0] + 1, "hps",
                                lambda uu=u: p2_unit(p1a, 2 * uu, 2, 24 + 2 * uu))
                    return done

                for ch in range(6):
                    rdy = min(8 * ch + 7, NF)
                    for mt in range(2):
                        enq(rdy, "hps", lambda c=ch, k=mt: _after_p0(c, k))

                        def _q1(c=ch, k=mt):
                            ev = q1_unit(c, k)
                            q1_emitted[c] = q1_emitted.get(c, 0) + 1
                            if q1_emitted[c] == 2:
                                for j in range(4 * c, 4 * c + 4):
                                    for hf in range(2):
                                        enq(cur_t[0] + 1, "hps",
                                            lambda jj=j, h=hf: q2_unit(jj, h))
                            return ev
                        enq(rdy, "hps", _q1)
                for u in range(12):
                    n0 = 4 * u
                    nn = min(4, NF - n0)
                    rdy = max(max(NF - 1 - n, n - 1) for n in range(n0, n0 + nn)) + 1
                    for mt in range(2):
                        enq(rdy, "hps", lambda uu=u, k=mt: _after_p1(uu, k))

                # --- scan state tiles (fused across chains) ------------
                zr_sb = pp.tile([128, 2, 2, 2, BC], wdt, tag="zrsb")  # [gate,mt,ch,b]
                rm_fb = pp.tile([128, 2, 2, BC], wdt, tag="rmfb")

                def zr_prefill(t):
                    # A_z/A_r pulled into the zr bank; independent of the
                    # scan state, so it is emitted at the END of iteration
                    # t-1 (before the pumped head units) and runs in the
                    # me-combine window on the other zr buffer.
                    s_f, s_b = t, NF - t
                    ps = zr_p.tile([128, 2, 2, 2, BC], f32, tag="zr")
                    nc.tensor.matmul(
                        ps[:, :, :, 0, :], ident,
                        azr[:, :, :, s_f * BC : (s_f + 1) * BC],
                        start=True, stop=False)
                    nc.tensor.matmul(
                        ps[:, :, :, 1, :], ident,
                        azr[:, :, :, s_b * BC : (s_b + 1) * BC],
                        start=False, stop=False)
                    return ps

                def scan_iter(t, ps):
                    """Interleaved fwd+bwd GRU iteration t (fused chains).

                    src_f = t, src_b = 47-t; dst(t-1) == src(t) on a chain.
                    psum zr: [gate2, mt2, ch2, 64]; psum h: [mt2, ch2, 64].
                    Critical path: me(t-1) -> r mms -> tanh_r -> rm' ->
                    h rm-mms -> tanh_h -> zpmt -> me(t); everything else
                    (z gate, h me-part mms, ozs) runs off that path.
                    rm = (1+r')/2 * me is folded into the h matmuls:
                    WhB^T rm = WhB2^T(r' me) + WhB2^T me  (WhB2 = WhB/2,
                    prescaled host-side).
                    """
                    s_f, s_b = t, NF - t
                    s_ap = hist[:, :, t, :, :]
                    # r-gate state matmuls first: tanh_r is on the critical
                    # path, z is not.
                    gates = [(1, ur_s), (0, wzb_s)] if t > 0 else [(0, wzb_s)]
                    for gi, (g, w_s) in enumerate(gates):
                        for mt in range(2):
                            msl = slice(mt * 128, (mt + 1) * 128)
                            for kt in range(2):
                                nc.tensor.matmul(
                                    ps[:, g, mt, :, :], w_s[:, kt, msl],
                                    s_ap[:, kt, :, :],
                                    start=False,
                                    stop=(gi == len(gates) - 1 and mt == 1
                                          and kt == 1))
                    if t > 0:
                        nc.scalar.activation(zr_sb[:, 1], ps[:, 1], AF.Tanh)
                    # H bank: A_h prefill + the state me-part (off-path)
                    ps_h = h_p.tile([128, 2, 2, BC], f32, tag="hh")
                    nc.tensor.matmul(
                        ps_h[:, :, 0, :], ident,
                        ah[:, :, s_f * BC : (s_f + 1) * BC],
                        start=True, stop=False)
                    nc.tensor.matmul(
                        ps_h[:, :, 1, :], ident,
                        ah[:, :, s_b * BC : (s_b + 1) * BC],
                        start=False, stop=(t == 0))
                    if t > 0:
                        for mt in range(2):
                            msl = slice(mt * 128, (mt + 1) * 128)
                            for kt in range(2):
                                nc.tensor.matmul(
                                    ps_h[:, mt, :, :], whb_s[:, kt, msl],
                                    s_ap[:, kt, :, :],
                                    start=False, stop=False)
                    nc.scalar.activation(zr_sb[:, 0], ps[:, 0], AF.Tanh)
                    if t > 0:
                        # rm' = r' * me(t-1); h rm-part matmuls (on path);
                        # WhB^T rm = WhB2^T (r' me) + WhB2^T me, WhB2
                        # prescaled on the host.
                        nc.vector.tensor_mul(rm_fb, zr_sb[:, 1], s_ap)
                        for mt in range(2):
                            msl = slice(mt * 128, (mt + 1) * 128)
                            for kt in range(2):
                                nc.tensor.matmul(
                                    ps_h[:, mt, :, :], whb_s[:, kt, msl],
                                    rm_fb[:, kt, :, :],
                                    start=False, stop=(mt == 1 and kt == 1))

                    # off-path: zp = 0.5 z' + 0.5 ; ozs = (0.5 - 0.5 z') s
                    zp = st.tile([128, 2, 2, BC], wdt, tag="zp")
                    nc.vector.tensor_scalar(
                        out=zp, in0=zr_sb[:, 0], scalar1=0.5, scalar2=0.5,
                        op0=ALU.mult, op1=ALU.add)
                    on = st.tile([128, 2, 2, BC], wdt, tag="on")
                    nc.vector.tensor_scalar(
                        out=on, in0=zr_sb[:, 0], scalar1=-0.5, scalar2=0.5,
                        op0=ALU.mult, op1=ALU.add)
                    ozs = st.tile([128, 2, 2, BC], wdt, tag="ozs")
                    nc.vector.tensor_mul(ozs, on, s_ap)
                    # on path: mt = tanh(psum_h); me = zp*mt + ozs
                    mt_sb = st.tile([128, 2, 2, BC], wdt, tag="mts")
                    nc.scalar.activation(mt_sb, ps_h, AF.Tanh)
                    zpmt = st.tile([128, 2, 2, BC], wdt, tag="zpmt")
                    nc.vector.tensor_mul(zpmt, zp, mt_sb)
                    nc.vector.tensor_add(hist[:, :, t + 1, :, :], zpmt, ozs)
                    # h_v adds ready at t: hbw[n] = m_b[slot 47-n] + m_f[slot n]
                    for n in range(NF):
                        if max(NF - 1 - n, n - 1) == t:
                            nc.vector.tensor_add(
                                hbw[:, :, n, :],
                                hist[:, :, QBLK - 1 - n, 1, :],
                                hist[:, :, n, 0, :])

                for t in range(NF):
                    cur_t[0] = t
                    scan_iter(t, zr_prefill(t))
                    pump(t, nh=3)
                t = NF
                while pending or evacs_next:
                    cur_t[0] = t
                    pump(t, nh=3)
                    t += 1

                # --- tail: combine vocab-half softmax partials, BCE ----
                sume_c = sp.tile([128, 24], f32, tag="sumc")
                nc.vector.tensor_add(sume_c, sume_acc[:, :24], sume_acc[:, 24:])
                qt_c = sp.tile([128, 24], f32, tag="qtc")
                nc.vector.tensor_add(qt_c, qt_acc[:, :24], qt_acc[:, 24:])
                rmax_c = sp.tile([128, 24], f32, tag="rmc")
                nc.vector.tensor_tensor(
                    out=rmax_c, in0=rmax_acc[:, :24], in1=rmax_acc[:, 24:],
                    op=ALU.max)
                qc_t = sp.tile([128, 24], f32, tag="qct")
                nc.vector.tensor_tensor(
                    out=qc_t, in0=qt_c, in1=rmax_c, op=ALU.is_ge)
                ab_t = sp.tile([128, 48], f32, tag="abt")
                nc.scalar.activation(ab_t, p_sb, AF.Abs)
                en_t = sp.tile([128, 48], f32, tag="ent")
                nc.scalar.activation(en_t, ab_t, AF.Exp, scale=-1.0)
                rl_t = sp.tile([128, 48], f32, tag="rlt")
                nc.scalar.activation(rl_t, p_sb, AF.Relu)
                l1p_t = sp.tile([128, 48], f32, tag="l1p")
                nc.scalar.activation(l1p_t, en_t, AF.Ln, bias=1.0)
                lse_t = sp.tile([128, 24], f32, tag="lse")
                nc.scalar.activation(lse_t, sume_c, AF.Ln)

                sp_t = sp.tile([128, 48], f32, tag="spt")
                nc.vector.tensor_add(sp_t, l1p_t, rl_t)
                ptt = sp.tile([128, 48], f32, tag="ptt")
                nc.vector.tensor_mul(ptt, p_sb, ptgt_s)
                bce = sp.tile([128, 48], f32, tag="bce")
                nc.vector.tensor_sub(bce, sp_t, ptt)
                nc.vector.reduce_sum(outp_s[:, 0:1], bce, axis=AX.X)
                gtz = sp.tile([128, 48], f32, tag="gtz")
                nc.vector.tensor_scalar(
                    out=gtz, in0=p_sb, scalar1=0.0, scalar2=None, op0=ALU.is_gt)
                pcr = sp.tile([128, 48], f32, tag="pcr")
                nc.vector.tensor_tensor(out=pcr, in0=gtz, in1=ptgt_s,
                                        op=ALU.is_equal)
                nc.vector.reduce_sum(outp_s[:, 1:2], pcr, axis=AX.X)
                nc.vector.reduce_sum(outp_s[:, 2:3], lse_t, axis=AX.X)
                nc.vector.reduce_sum(outp_s[:, 3:4], qt_c, axis=AX.X)
                nc.vector.reduce_sum(outp_s[:, 4:5], qc_t, axis=AX.X)
            nc.sync.dma_start(out=outp[:], in_=outp_s)

    nc.finalize()
    return nc


def _get_nc(wob_nonzero: bool):
    key = ("nc", wob_nonzero, BF16)
    if key not in _CACHE:
        _CACHE[key] = _build(wob_nonzero)
    return _CACHE[key]


def _wdt_np():
    if BF16:
        import ml_dtypes

        return ml_dtypes.bfloat16
    return np.float32


def _prep_inputs(inputs):
    f = lambda k: np.ascontiguousarray(np.asarray(inputs[k]), dtype=np.float32)
    wdt = _wdt_np()
    w = lambda a: np.ascontiguousarray(a).astype(wdt)
    wid = np.asarray(inputs["wid"]).astype(np.int64).reshape(B, N)
    tree_vec = f("tree_vec")
    Wz, bz = f("Wz"), f("bz")
    Wr_, Ur_, br = f("Wr"), f("Ur"), f("br")
    Wh, bh = f("Wh"), f("bh")
    W_w, W_b = f("W_w"), f("W_b")
    U_w, U_b = f("U_w"), f("U_b")
    Wo_w, Wo_b = f("Wo_w"), f("Wo_b")
    Us_w, Us_b = f("Us_w"), f("Us_b")
    emb = f("embedding")

    def c2(v):
        return np.ascontiguousarray(v.reshape(2, 128).T)

    # sigma(x) = (1+tanh(x/2))/2: z and r pre-activations halved host-side.
    # WhB is halved too: WhB^T rm = (WhB/2)^T (r' me) + (WhB/2)^T me.
    shared = dict(
        emb=w(emb),
        WzT=w(0.5 * Wz[:H]), WzB=w(0.5 * Wz[H:]),
        WhT=w(Wh[:H]), WhB=w(0.5 * Wh[H:]),
        Wr=w(0.5 * Wr_), Ur=w(0.5 * Ur_),
        UwX=w(U_w[:H]), UwH=w(U_w[H : 2 * H]), UwL=w(U_w[2 * H :]),
        WwH=w(W_w[:H]), WwL=w(W_w[H:]),
        Wo=w(Wo_w), Us=w(Us_w),
        bz2=c2(0.5 * bz), bh2=c2(bh), br2=c2(0.5 * br),
        ub2=c2(U_b), wb2=c2(W_b),
        usb=np.full((128, 1), float(Us_b.reshape(-1)[0]), np.float32),
    )
    wob_nonzero = bool(np.any(Wo_b != 0))
    if wob_nonzero:
        shared["wob"] = Wo_b.reshape(1, V)

    ii, pprt = np.meshgrid(np.arange(48), np.arange(128), indexing="xy")
    tblk = 2 * ii + pprt // 64
    ptgt = np.ascontiguousarray((tblk <= 46).astype(np.float32))

    in_maps = []
    for c in range(NC):
        w2 = wid[c * BC : (c + 1) * BC]
        flat = np.ascontiguousarray(w2.T).reshape(-1)
        m = dict(shared)
        m["gidx"] = np.ascontiguousarray(flat.reshape(24, 128)).astype(np.int32)
        m["tvt"] = np.ascontiguousarray(
            np.tile(tree_vec[c * BC : (c + 1) * BC].T, (1, 8))
        ).astype(wdt)
        m["qtgt"] = np.ascontiguousarray(flat.reshape(24, 128).T).astype(np.float32)
        m["ptgt"] = ptgt
        in_maps.append(m)
    return in_maps, wob_nonzero, float(Us_b.reshape(-1)[0])


def _combine(results, us_b):
    S = np.zeros(8, np.float64)
    for r in results:
        S += np.asarray(r["outp"], np.float64).sum(axis=0)
    pad_bce = max(us_b, 0.0) + np.log1p(np.exp(-abs(us_b)))
    pad_corr = 1.0 if us_b <= 0 else 0.0
    n_pad = NC * (PPAD - PROWS)
    p_loss = (S[0] - n_pad * pad_bce) / B
    p_acc = (S[1] - n_pad * pad_corr) / (PBLK * B)
    q_loss = (S[2] - S[3]) / B
    q_acc = S[4] / (QBLK * B)
    return np.array([q_loss, p_loss, q_acc, p_acc], np.float32)


def kernel(**inputs) -> np.ndarray:
    from concourse.bass_utils import run_bass_kernel_spmd

    in_maps, wob_nonzero, us_b = _prep_inputs(inputs)
    nc = _get_nc(wob_nonzero)
    res = run_bass_kernel_spmd(nc, in_maps, list(range(NC)))
    return _combine(res.results, us_b)


# revision 67
# speedup vs baseline: 1.4747x; 1.4747x over previous
"""DGLJTNNDecoder kernel for 8x Trainium2 NeuronCores (Bass/Tile), v3.

Tree-GRU decoder over B=512 chain-trees (N=48 nodes), T=94 DFS steps,
followed by two MLP heads producing (q_loss, p_loss, q_acc, p_acc).
Data-parallel over trees, 64 trees per core; host combines partials.

v3 highlights:
  - sigmoids as scaled tanh (sigma(x) = (1+tanh(x/2))/2, halving folded
    into host-prescaled Wz/bz/Wr/Ur/br) so everything up to the tail
    runs out of the exp_and_others ACT table set (tanh/exp/relu/
    identity/abs); a single table switch at the tail covers the Ln ops.
  - the fwd and bwd DFS chains share one interleaved history tile
    hist[128, kt, slot, chain, 64]: slot t holds exactly the two state
    vectors iteration t consumes, so every state matmul runs at N=128
    with one LDWEIGHTS per weight tile, and all scan elementwise/tanh
    ops are fused across chains (FD=256).
  - A_z and A_r live interleaved in one arena so a single N=256
    identity-matmul per chain prefills the z+r PSUM bank (biases are
    folded into the projections in phase B).
  - h_v is accumulated in node order: hbw[n] = m_bwd[slot 47-n] +
    m_fwd[slot n]  (also fixes v1's off-by-one).
  - gathers/weight DMAs fan out over four trigger queues and the
    embedding transposes chase them, so the prefix is short; head
    matmuls (phase-B remainder, p0/p1/q1/q2/p2) are pumped between
    scan iterations with psum evacuations deferred one iteration.
"""

import sys

if "/opt/trn_rl_repo" not in sys.path:
    sys.path.insert(0, "/opt/trn_rl_repo")

import numpy as np

B, N, H, L, V = 512, 48, 256, 64, 800
NC = 8
BC = B // NC            # 64 trees per core
NF = N - 1              # 47 forward steps (= backward steps)
T = 2 * NF              # 94
NODES = N * BC          # 3072 gathered node rows per core
QBLK = NF + 1           # 48 q-head blocks
PBLK = T + 1            # 95 p-head blocks
PROWS = PBLK * BC       # 6080
PPAD = 48 * 128         # 6144

BF16 = True

_CACHE = {}


def _build(wob_nonzero: bool):
    import concourse.bass as bass
    import concourse.tile as tile
    from concourse import bacc, mybir
    from concourse.masks import make_identity

    f32 = mybir.dt.float32
    i32 = mybir.dt.int32
    wdt = mybir.dt.bfloat16 if BF16 else f32
    AF = mybir.ActivationFunctionType
    ALU = mybir.AluOpType
    AX = mybir.AxisListType

    nc = bacc.Bacc()

    def din(name, shape, dtype=f32):
        return nc.declare_dram_parameter(name, list(shape), dtype, isOutput=False)

    gidx = din("gidx", [24, 128], i32)
    tvt = din("tvt", [L, 8 * BC], wdt)
    qtgt = din("qtgt", [128, 24])
    ptgt = din("ptgt", [128, 48])
    emb = din("emb", [V, H], wdt)
    WzT = din("WzT", [H, H], wdt); WzB = din("WzB", [H, H], wdt)
    WhT = din("WhT", [H, H], wdt); WhB = din("WhB", [H, H], wdt)
    Wr = din("Wr", [H, H], wdt); Ur = din("Ur", [H, H], wdt)
    UwX = din("UwX", [H, H], wdt); UwH = din("UwH", [H, H], wdt)
    UwL = din("UwL", [L, H], wdt)
    WwH = din("WwH", [H, H], wdt); WwL = din("WwL", [L, H], wdt)
    Wo = din("Wo", [H, V], wdt); Us = din("Us", [H, 1], wdt)
    bz2 = din("bz2", [128, 2]); bh2 = din("bh2", [128, 2]); br2 = din("br2", [128, 2])
    ub2 = din("ub2", [128, 2]); wb2 = din("wb2", [128, 2])
    usb = din("usb", [128, 1])
    wob = din("wob", [1, V]) if wob_nonzero else None
    outp = nc.declare_dram_parameter("outp", [128, 8], f32, isOutput=True)

    def rearr2(ap):
        return ap.rearrange("(k p) m -> p k m", p=128)

    with tile.TileContext(nc) as tc:
        with (
            tc.tile_pool(name="persist", bufs=1) as pp,
            tc.tile_pool(name="small", bufs=1) as sp,
        ):
            # --- DMA triggers fan out over queues (gpsimd reserved for
            # the indirect gathers, which only it can issue) --------------
            dma_queues = [nc.sync, nc.scalar]
            dq_i = [0]

            def dma_next(out, in_):
                eng = dma_queues[dq_i[0] % len(dma_queues)]
                dq_i[0] += 1
                eng.dma_start(out=out, in_=in_)

            idx_s = pp.tile([128, 24], i32, tag="idx")
            nc.sync.dma_start(out=idx_s, in_=gidx[:].rearrange("c p -> p c"))

            def loadw(dram, shape, tag, dt=wdt, re2=True):
                t = pp.tile(shape, dt, tag=tag)
                dma_next(t, rearr2(dram[:]) if re2 else dram[:])
                return t

            wzb_s = loadw(WzB, [128, 2, H], "wzb")
            whb_s = loadw(WhB, [128, 2, H], "whb")
            ur_s = loadw(Ur, [128, 2, H], "ur")
            wzt_s = loadw(WzT, [128, 2, H], "wzt")
            wht_s = loadw(WhT, [128, 2, H], "wht")
            wr_s = loadw(Wr, [128, 2, H], "wr")
            uwx_s = loadw(UwX, [128, 2, H], "uwx")
            uwh_s = loadw(UwH, [128, 2, H], "uwh")
            wwh_s = loadw(WwH, [128, 2, H], "wwh")
            wo_s = loadw(Wo, [128, 2, V], "wo")
            us_s = loadw(Us, [128, 2, 1], "us")
            uwl_s = loadw(UwL, [L, H], "uwl", re2=False)
            wwl_s = loadw(WwL, [L, H], "wwl", re2=False)
            bz_s = loadw(bz2, [128, 2], "bz", dt=f32, re2=False)
            bh_s = loadw(bh2, [128, 2], "bh", dt=f32, re2=False)
            br_s = loadw(br2, [128, 2], "br", dt=f32, re2=False)
            ub_s = loadw(ub2, [128, 2], "ub", dt=f32, re2=False)
            wb_s = loadw(wb2, [128, 2], "wb", dt=f32, re2=False)
            usb_s = loadw(usb, [128, 1], "usb", dt=f32, re2=False)
            qtgt_s = loadw(qtgt, [128, 24], "qtgt", dt=f32, re2=False)
            ptgt_s = loadw(ptgt, [128, 48], "ptgt", dt=f32, re2=False)
            wob_s = loadw(wob, [1, V], "wob", dt=f32, re2=False) if wob_nonzero else None

            tvrep = pp.tile([L, 8, BC], wdt, tag="tvrep")
            dma_next(tvrep, tvt[:].rearrange("l (r b) -> l r b", b=BC))

            ident = pp.tile([128, 128], wdt, tag="ident")
            make_identity(nc, ident)

            iota_f = pp.tile([128, V], f32, tag="iotaf")
            iota_i = pp.tile([128, V], i32, tag="iotai")
            nc.gpsimd.iota(iota_i, pattern=[[1, V]], base=0, channel_multiplier=0)
            nc.vector.tensor_copy(iota_f, iota_i)

            # persistent tensors
            xt = pp.tile([128, 2, NODES], wdt, tag="xt")
            hist = pp.tile([128, 2, QBLK, 2, BC], wdt, tag="hist")  # [kt,slot,ch,b]
            hbw = pp.tile([128, 2, QBLK, BC], wdt, tag="hbw")       # node order
            azr = pp.tile([128, 2, 2, NODES], wdt, tag="azr")       # [gate(z,r),mt,*]
            ah = pp.tile([128, 2, NODES], wdt, tag="ah")
            p0a = pp.tile([128, 2, NODES], wdt, tag="p0a")
            p1a = pp.tile([128, 2, NODES], wdt, tag="p1a")
            q1a = pp.tile([128, 2, NODES], wdt, tag="q1a")
            scr = pp.tile([128, 1024], f32, tag="scr")
            scr2 = pp.tile([128, 1024], f32, tag="scr2")

            nc.vector.memset(hist[:, :, 0, :, :], 0.0)
            nc.vector.memset(p1a[:, :, NF * BC:], 0.0)

            outp_s = sp.tile([128, 8], f32, tag="outp")
            nc.vector.memset(outp_s, 0.0)
            sume_acc = sp.tile([128, 48], f32, tag="sume")
            qt_acc = sp.tile([128, 48], f32, tag="qta")
            rmax_acc = sp.tile([128, 48], f32, tag="rmx")

            hbw_f = hbw.rearrange("p k s b -> p k (s b)")
            xt_v = xt.rearrange("p k (c f) -> p k c f", f=128)

            # mfq-equivalent strided views of hist (fwd chain, ch=0):
            def mfq_cols(kt, s0, ns):
                return hist[:, kt, s0 : s0 + ns, 0, :]

            # ================= scan-era pools ==========================
            # PSUM (8 banks): tps x1 + zr x2 + h x1 + head-pipe x4.
            with (
                tc.tile_pool(name="gath", bufs=1) as gp,
                tc.tile_pool(name="tps", bufs=1, space="PSUM") as tpp,
                tc.tile_pool(name="zrp", bufs=2, space="PSUM") as zr_p,
                tc.tile_pool(name="hp", bufs=1, space="PSUM") as h_p,
                tc.tile_pool(name="hps", bufs=4, space="PSUM") as hps_p,
                tc.tile_pool(name="st", bufs=4) as st,
            ):
                # --- Phase A: all 24 gathers issue back-to-back on the
                # gpsimd queue (distinct buffers, no tag reuse); the
                # transposes chase them - the first four pairs feed the
                # prefix phase-B chunks, the rest are pumped as era units.
                gather_order = [20, 21, 22, 23, 0, 1, 2, 3,
                                16, 17, 18, 19, 4, 5, 6, 7,
                                12, 13, 14, 15, 8, 9, 10, 11]
                xg_tiles = {}
                for c in gather_order:
                    xg = gp.tile([128, H], wdt, tag=f"xg{c}")
                    nc.gpsimd.indirect_dma_start(
                        out=xg, out_offset=None, in_=emb[:],
                        in_offset=bass.IndirectOffsetOnAxis(
                            ap=idx_s[:, c : c + 1], axis=0))
                    xg_tiles[c] = xg

                evac_seq = [0]

                def transpose_pair(c0):
                    pt = tpp.tile([128, 2, 2, 128], wdt, tag="pt")
                    for i, c in enumerate((c0, c0 + 1)):
                        for ht in range(2):
                            nc.tensor.transpose(
                                pt[:, ht, i, :],
                                xg_tiles[c][:, ht * 128 : (ht + 1) * 128], ident)

                    def evac():
                        evac_seq[0] ^= 1
                        if evac_seq[0]:
                            nc.scalar.copy(xt_v[:, :, c0 : c0 + 2, :], pt)
                        else:
                            nc.vector.tensor_copy(xt_v[:, :, c0 : c0 + 2, :], pt)
                    return evac

                # warm the PE clock (HAM) while the first gathers land:
                # harmless matmuls on the identity into a scratch bank.
                warm = tpp.tile([128, 512], f32, tag="pt")
                for i in range(24):
                    nc.tensor.matmul(
                        warm[:, :128], ident, ident,
                        start=(i == 0), stop=(i == 23))

                for c0 in (20, 22, 0, 2):
                    transpose_pair(c0)()

                # --- evac engine alternation (2:1 toward ACT; DVE is the
                # hotter engine during the era) --------------------------
                eng_flip = [0]

                def _evac_on_act():
                    eng_flip[0] = (eng_flip[0] + 1) % 3
                    return eng_flip[0] != 0

                def evac_relu(dst, ps, bias_ap):
                    if _evac_on_act():
                        nc.scalar.activation(dst, ps, AF.Relu, bias=bias_ap)
                    else:
                        nc.vector.tensor_scalar(
                            out=dst, in0=ps, scalar1=bias_ap, scalar2=0.0,
                            op0=ALU.add, op1=ALU.max)

                def evac_ident(dst, ps, bias_ap):
                    if _evac_on_act():
                        nc.scalar.activation(dst, ps, AF.Identity, bias=bias_ap)
                    else:
                        nc.vector.tensor_scalar(
                            out=dst, in0=ps, scalar1=bias_ap, scalar2=None,
                            op0=ALU.add)

                # --- units ---------------------------------------------
                def b_unit(mat, ch, mt):
                    w_s, b_s = {
                        "z": (wzt_s, bz_s), "h": (wht_s, bh_s),
                        "r": (wr_s, br_s)}[mat]
                    dst = {"z": azr[:, 0], "h": ah, "r": azr[:, 1]}[mat]
                    msl = slice(mt * 128, (mt + 1) * 128)
                    csl = slice(ch * 512, (ch + 1) * 512)
                    ps = hps_p.tile([128, 512], f32, tag="hps")
                    for kt in range(2):
                        nc.tensor.matmul(
                            ps, w_s[:, kt, msl], xt[:, kt, csl],
                            start=(kt == 0), stop=(kt == 1))
                    return lambda: evac_ident(dst[:, mt, csl], ps, b_s[:, mt:mt+1])

                def p0_unit(ch, mt):
                    msl = slice(mt * 128, (mt + 1) * 128)
                    csl = slice(ch * 512, (ch + 1) * 512)
                    ps = hps_p.tile([128, 512], f32, tag="hps")
                    for kt in range(2):
                        nc.tensor.matmul(
                            ps, uwx_s[:, kt, msl], xt[:, kt, csl],
                            start=(kt == 0), stop=False)
                    for kt in range(2):
                        nc.tensor.matmul(
                            ps, uwh_s[:, kt, msl], mfq_cols(kt, 8 * ch, 8),
                            start=False, stop=False)
                    nc.tensor.matmul(
                        ps, uwl_s[:, msl], tvrep[:, :8, :],
                        start=False, stop=True)
                    return lambda: evac_relu(p0a[:, mt, csl], ps, ub_s[:, mt:mt+1])

                def p1_unit(u, mt):
                    n0 = 4 * u
                    nn = min(4, NF - n0)
                    cw = nn * BC
                    msl = slice(mt * 128, (mt + 1) * 128)
                    csl = slice(n0 * BC, n0 * BC + cw)
                    ps = hps_p.tile([128, 512], f32, tag="hps")
                    psv = ps[:, :cw]
                    for kt in range(2):
                        nc.tensor.matmul(
                            psv, uwx_s[:, kt, msl], xt[:, kt, csl],
                            start=(kt == 0), stop=False)
                    for kt in range(2):
                        nc.tensor.matmul(
                            psv, uwh_s[:, kt, msl], hbw_f[:, kt, csl],
                            start=False, stop=False)
                    nc.tensor.matmul(
                        psv, uwl_s[:, msl], tvrep[:, :nn, :],
                        start=False, stop=True)
                    return lambda: evac_relu(p1a[:, mt, csl], psv, ub_s[:, mt:mt+1])

                def q1_unit(ch, mt):
                    msl = slice(mt * 128, (mt + 1) * 128)
                    csl = slice(ch * 512, (ch + 1) * 512)
                    ps = hps_p.tile([128, 512], f32, tag="hps")
                    for kt in range(2):
                        nc.tensor.matmul(
                            ps, wwh_s[:, kt, msl], mfq_cols(kt, 8 * ch, 8),
                            start=(kt == 0), stop=False)
                    nc.tensor.matmul(
                        ps, wwl_s[:, msl], tvrep[:, :8, :],
                        start=False, stop=True)
                    return lambda: evac_relu(q1a[:, mt, csl], ps, wb_s[:, mt:mt+1])

                def q2_unit(j, half):
                    # vocab half of one logits row-tile through the shared
                    # head-pipe pool; softmax partials (sume/qt/rmax) are
                    # accumulated per half and combined in three tail ops.
                    n0, nn = (0, 512) if half == 0 else (512, V - 512)
                    col = half * 24 + j
                    psq = hps_p.tile([128, 512], f32, tag="hps")
                    pv = psq[:, :nn]
                    jsl = slice(j * 128, (j + 1) * 128)
                    for kt in range(2):
                        nc.tensor.matmul(
                            pv, q1a[:, kt, jsl], wo_s[:, kt, n0 : n0 + nn],
                            start=(kt == 0), stop=(kt == 1))
                    if wob_nonzero:
                        wv = wob_s[:]
                        wb_b = bass.AP(
                            tensor=wv.tensor, offset=wv.offset + n0,
                            ap=[[0, 128], [1, nn]])
                        nc.vector.tensor_add(pv, pv, wb_b)

                    def softmax():
                        nc.scalar.activation(
                            scr[:, :nn], pv, AF.Exp,
                            accum_out=sume_acc[:, col : col + 1])
                        nc.vector.scalar_tensor_tensor(
                            out=scr2[:, :nn], in0=iota_f[:, n0 : n0 + nn],
                            scalar=qtgt_s[:, j : j + 1],
                            in1=pv, op0=ALU.is_equal, op1=ALU.mult,
                            accum_out=qt_acc[:, col : col + 1])
                        nc.vector.reduce_max(
                            rmax_acc[:, col : col + 1], pv, axis=AX.X)
                    return softmax

                # p2 sub-units: a few p1-row tiles reduced against Us as
                # soon as their relu arena columns are complete, straight
                # into the matching p_sb slice (no persistent psum).
                p_sb = sp.tile([128, 48], f32, tag="psb")

                def p2_unit(src, j0, nj, col0):
                    ps = hps_p.tile([128, 512], f32, tag="hps")
                    for j in range(j0, j0 + nj):
                        for kt in range(2):
                            nc.tensor.matmul(
                                ps[:, j - j0 : j - j0 + 1],
                                src[:, kt, j * 128 : (j + 1) * 128],
                                us_s[:, kt, :],
                                start=(kt == 0), stop=(kt == 1))
                    return lambda: nc.scalar.activation(
                        p_sb[:, col0 : col0 + nj], ps[:, :nj],
                        AF.Identity, bias=usb_s[:, 0:1])

                # --- pump scheduler ------------------------------------
                pending = []
                seq_ctr = [0]

                def enq(ready, kind, fn, deadline=10**9):
                    pending.append([ready, seq_ctr[0], kind, deadline, fn])
                    seq_ctr[0] += 1

                evacs_next = []
                q1_emitted = {}
                p0_emitted = {}
                p1_emitted = {}
                cur_t = [0]

                def flush_evacs():
                    for ev in evacs_next:
                        ev()
                    evacs_next.clear()

                def pump(t, nh=3):
                    flush_evacs()
                    budget = {"hps": nh, "tps": 1}
                    pending.sort(key=lambda u: (u[0], u[1]))
                    for u in list(pending):
                        ready, _, kind, deadline, fn = u
                        if ready > t or budget[kind] == 0:
                            continue
                        assert t <= deadline, f"unit past deadline at iter {t}"
                        budget[kind] -= 1
                        pending.remove(u)
                        evacs_next.append(fn())

                # phase-B chunks 5,0 then 4,1 in the prefix (the scan
                # reads them by iters 0 and 8); software-pipelined evacs
                # (hps bufs=2 -> <=2 in flight).
                def b_prefix(chunks):
                    prev_ev = None
                    for ch in chunks:
                        for mat in ("z", "r", "h"):
                            for mt in range(2):
                                ev = b_unit(mat, ch, mt)
                                if prev_ev is not None:
                                    prev_ev()
                                prev_ev = ev
                    prev_ev()

                b_prefix((5, 0))
                for c0 in (16, 18, 4, 6):
                    transpose_pair(c0)()
                b_prefix((4, 1))
                # the last four transpose pairs are era units: pair at
                # pump p writes xt at pump p+1 (evacs run first).
                for c0, p in ((12, 0), (14, 1), (8, 2), (10, 3)):
                    enq(p, "tps", lambda cc=c0: transpose_pair(cc), deadline=p)
                # remaining B chunks (first read at scan iter 16 ->
                # emission deadline 14); ready gates on their transposes.
                for ch, rdy, dl in ((3, 2, 16), (2, 4, 16)):
                    for mat in ("z", "r", "h"):
                        for mt in range(2):
                            enq(rdy, "hps",
                                lambda m=mat, c=ch, k=mt: b_unit(m, c, k),
                                deadline=dl - 2)

                def _after_p0(c, k):
                    ev = p0_unit(c, k)
                    def done():
                        ev()
                        p0_emitted[c] = p0_emitted.get(c, 0) + 1
                        if p0_emitted[c] == 2:
                            # p0a chunk c complete -> p-row tiles 4c..4c+3
                            enq(cur_t[0] + 1, "hps",
                                lambda cc=c: p2_unit(p0a, 4 * cc, 4, 4 * cc))
                    return done

                def _after_p1(u, k):
                    ev = p1_unit(u, k)
                    def done():
                        ev()
                        p1_emitted[u] = p1_emitted.get(u, 0) + 1
                        if p1_emitted[u] == 2:
                            # p1a nodes 4u..4u+3 complete -> tiles 2u,2u+1
                            enq(cur_t[0] + 1, "hps",
                                lambda uu=u: p2_unit(p1a, 2 * uu, 2, 24 + 2 * uu))
                    return done

                for ch in range(6):
                    rdy = min(8 * ch + 7, NF)
                    for mt in range(2):
                        enq(rdy, "hps", lambda c=ch, k=mt: _after_p0(c, k))

                        def _q1(c=ch, k=mt):
                            ev = q1_unit(c, k)
                            q1_emitted[c] = q1_emitted.get(c, 0) + 1
                            if q1_emitted[c] == 2:
                                for j in range(4 * c, 4 * c + 4):
                                    for hf in range(2):
                                        enq(cur_t[0] + 1, "hps",
                                            lambda jj=j, h=hf: q2_unit(jj, h))
                            return ev
                        enq(rdy, "hps", _q1)
                for u in range(12):
                    n0 = 4 * u
                    nn = min(4, NF - n0)
                    rdy = max(max(NF - 1 - n, n - 1) for n in range(n0, n0 + nn)) + 1
                    for mt in range(2):
                        enq(rdy, "hps", lambda uu=u, k=mt: _after_p1(uu, k))

                # --- scan state tiles (fused across chains) ------------
                zr_sb = pp.tile([128, 2, 2, 2, BC], wdt, tag="zrsb")  # [gate,mt,ch,b]
                rm_fb = pp.tile([128, 2, 2, BC], wdt, tag="rmfb")

                def zr_prefill(t):
                    # A_z/A_r pulled into the zr bank; independent of the
                    # scan state, so it is emitted at the END of iteration
                    # t-1 (before the pumped head units) and runs in the
                    # me-combine window on the other zr buffer.
                    s_f, s_b = t, NF - t
                    ps = zr_p.tile([128, 2, 2, 2, BC], f32, tag="zr")
                    nc.tensor.matmul(
                        ps[:, :, :, 0, :], ident,
                        azr[:, :, :, s_f * BC : (s_f + 1) * BC],
                        start=True, stop=False)
                    nc.tensor.matmul(
                        ps[:, :, :, 1, :], ident,
                        azr[:, :, :, s_b * BC : (s_b + 1) * BC],
                        start=False, stop=False)
                    return ps

                def scan_iter(t, ps):
                    """Interleaved fwd+bwd GRU iteration t (fused chains).

                    src_f = t, src_b = 47-t; dst(t-1) == src(t) on a chain.
                    psum zr: [gate2, mt2, ch2, 64]; psum h: [mt2, ch2, 64].
                    Critical path: me(t-1) -> r mms -> tanh_r -> rm' ->
                    h rm-mms -> tanh_h -> zpmt -> me(t); everything else
                    (z gate, h me-part mms, ozs) runs off that path.
                    rm = (1+r')/2 * me is folded into the h matmuls:
                    WhB^T rm = WhB2^T(r' me) + WhB2^T me  (WhB2 = WhB/2,
                    prescaled host-side).
                    """
                    s_f, s_b = t, NF - t
                    s_ap = hist[:, :, t, :, :]
                    # r-gate state matmuls first: tanh_r is on the critical
                    # path, z is not.
                    gates = [(1, ur_s), (0, wzb_s)] if t > 0 else [(0, wzb_s)]
                    for gi, (g, w_s) in enumerate(gates):
                        for mt in range(2):
                            msl = slice(mt * 128, (mt + 1) * 128)
                            for kt in range(2):
                                nc.tensor.matmul(
                                    ps[:, g, mt, :, :], w_s[:, kt, msl],
                                    s_ap[:, kt, :, :],
                                    start=False,
                                    stop=(gi == len(gates) - 1 and mt == 1
                                          and kt == 1))
                    if t > 0:
                        nc.scalar.activation(zr_sb[:, 1], ps[:, 1], AF.Tanh)
                    # H bank: A_h prefill + the state me-part (off-path)
                    ps_h = h_p.tile([128, 2, 2, BC], f32, tag="hh")
                    nc.tensor.matmul(
                        ps_h[:, :, 0, :], ident,
                        ah[:, :, s_f * BC : (s_f + 1) * BC],
                        start=True, stop=False)
                    nc.tensor.matmul(
                        ps_h[:, :, 1, :], ident,
                        ah[:, :, s_b * BC : (s_b + 1) * BC],
                        start=False, stop=(t == 0))
                    if t > 0:
                        for mt in range(2):
                            msl = slice(mt * 128, (mt + 1) * 128)
                            for kt in range(2):
                                nc.tensor.matmul(
                                    ps_h[:, mt, :, :], whb_s[:, kt, msl],
                                    s_ap[:, kt, :, :],
                                    start=False, stop=False)
                    nc.scalar.activation(zr_sb[:, 0], ps[:, 0], AF.Tanh)
                    if t > 0:
                        # rm' = r' * me(t-1); h rm-part matmuls (on path);
                        # WhB^T rm = WhB2^T (r' me) + WhB2^T me, WhB2
                        # prescaled on the host.
                        nc.vector.tensor_mul(rm_fb, zr_sb[:, 1], s_ap)
                        for mt in range(2):
                            msl = slice(mt * 128, (mt + 1) * 128)
                            for kt in range(2):
                                nc.tensor.matmul(
                                    ps_h[:, mt, :, :], whb_s[:, kt, msl],
                                    rm_fb[:, kt, :, :],
                                    start=False, stop=(mt == 1 and kt == 1))

                    # off-path: zp = 0.5 z' + 0.5 ; ozs = (0.5 - 0.5 z') s
                    zp = st.tile([128, 2, 2, BC], wdt, tag="zp")
                    nc.vector.tensor_scalar(
                        out=zp, in0=zr_sb[:, 0], scalar1=0.5, scalar2=0.5,
                        op0=ALU.mult, op1=ALU.add)
                    on = st.tile([128, 2, 2, BC], wdt, tag="on")
                    nc.vector.tensor_scalar(
                        out=on, in0=zr_sb[:, 0], scalar1=-0.5, scalar2=0.5,
                        op0=ALU.mult, op1=ALU.add)
                    ozs = st.tile([128, 2, 2, BC], wdt, tag="ozs")
                    nc.vector.tensor_mul(ozs, on, s_ap)
                    # on path: mt = tanh(psum_h); me = zp*mt + ozs
                    mt_sb = st.tile([128, 2, 2, BC], wdt, tag="mts")
                    nc.scalar.activation(mt_sb, ps_h, AF.Tanh)
                    zpmt = st.tile([128, 2, 2, BC], wdt, tag="zpmt")
                    nc.vector.tensor_mul(zpmt, zp, mt_sb)
                    nc.vector.tensor_add(hist[:, :, t + 1, :, :], zpmt, ozs)
                    # h_v adds ready at t: hbw[n] = m_b[slot 47-n] + m_f[slot n]
                    for n in range(NF):
                        if max(NF - 1 - n, n - 1) == t:
                            nc.vector.tensor_add(
                                hbw[:, :, n, :],
                                hist[:, :, QBLK - 1 - n, 1, :],
                                hist[:, :, n, 0, :])

                for t in range(NF):
                    cur_t[0] = t
                    scan_iter(t, zr_prefill(t))
                    pump(t, nh=3)
                t = NF
                while pending or evacs_next:
                    cur_t[0] = t
                    pump(t, nh=3)
                    t += 1

                # --- tail: combine vocab-half softmax partials, BCE ----
                sume_c = sp.tile([128, 24], f32, tag="sumc")
                nc.vector.tensor_add(sume_c, sume_acc[:, :24], sume_acc[:, 24:])
                qt_c = sp.tile([128, 24], f32, tag="qtc")
                nc.vector.tensor_add(qt_c, qt_acc[:, :24], qt_acc[:, 24:])
                rmax_c = sp.tile([128, 24], f32, tag="rmc")
                nc.vector.tensor_tensor(
                    out=rmax_c, in0=rmax_acc[:, :24], in1=rmax_acc[:, 24:],
                    op=ALU.max)
                qc_t = sp.tile([128, 24], f32, tag="qct")
                nc.vector.tensor_tensor(
                    out=qc_t, in0=qt_c, in1=rmax_c, op=ALU.is_ge)
                ab_t = sp.tile([128, 48], f32, tag="abt")
                nc.scalar.activation(ab_t, p_sb, AF.Abs)
                en_t = sp.tile([128, 48], f32, tag="ent")
                nc.scalar.activation(en_t, ab_t, AF.Exp, scale=-1.0)
                rl_t = sp.tile([128, 48], f32, tag="rlt")
                nc.scalar.activation(rl_t, p_sb, AF.Relu)
                l1p_t = sp.tile([128, 48], f32, tag="l1p")
                nc.scalar.activation(l1p_t, en_t, AF.Ln, bias=1.0)
                lse_t = sp.tile([128, 24], f32, tag="lse")
                nc.scalar.activation(lse_t, sume_c, AF.Ln)

                sp_t = sp.tile([128, 48], f32, tag="spt")
                nc.vector.tensor_add(sp_t, l1p_t, rl_t)
                ptt = sp.tile([128, 48], f32, tag="ptt")
                nc.vector.tensor_mul(ptt, p_sb, ptgt_s)
                bce = sp.tile([128, 48], f32, tag="bce")
                nc.vector.tensor_sub(bce, sp_t, ptt)
                nc.vector.reduce_sum(outp_s[:, 0:1], bce, axis=AX.X)
                gtz = sp.tile([128, 48], f32, tag="gtz")
                nc.vector.tensor_scalar(
                    out=gtz, in0=p_sb, scalar1=0.0, scalar2=None, op0=ALU.is_gt)
                pcr = sp.tile([128, 48], f32, tag="pcr")
                nc.vector.tensor_tensor(out=pcr, in0=gtz, in1=ptgt_s,
                                        op=ALU.is_equal)
                nc.vector.reduce_sum(outp_s[:, 1:2], pcr, axis=AX.X)
                nc.vector.reduce_sum(outp_s[:, 2:3], lse_t, axis=AX.X)
                nc.vector.reduce_sum(outp_s[:, 3:4], qt_c, axis=AX.X)
                nc.vector.reduce_sum(outp_s[:, 4:5], qc_t, axis=AX.X)
            nc.sync.dma_start(out=outp[:], in_=outp_s)

    nc.finalize()
    return nc


def _get_nc(wob_nonzero: bool):
    key = ("nc", wob_nonzero, BF16)
    if key not in _CACHE:
        _CACHE[key] = _build(wob_nonzero)
    return _CACHE[key]


def _wdt_np():
    if BF16:
        import ml_dtypes

        return ml_dtypes.bfloat16
    return np.float32


def _prep_inputs(inputs):
    f = lambda k: np.ascontiguousarray(np.asarray(inputs[k]), dtype=np.float32)
    wdt = _wdt_np()
    w = lambda a: np.ascontiguousarray(a).astype(wdt)
    wid = np.asarray(inputs["wid"]).astype(np.int64).reshape(B, N)
    tree_vec = f("tree_vec")
    Wz, bz = f("Wz"), f("bz")
    Wr_, Ur_, br = f("Wr"), f("Ur"), f("br")
    Wh, bh = f("Wh"), f("bh")
    W_w, W_b = f("W_w"), f("W_b")
    U_w, U_b = f("U_w"), f("U_b")
    Wo_w, Wo_b = f("Wo_w"), f("Wo_b")
    Us_w, Us_b = f("Us_w"), f("Us_b")
    emb = f("embedding")

    def c2(v):
        return np.ascontiguousarray(v.reshape(2, 128).T)

    # sigma(x) = (1+tanh(x/2))/2: z and r pre-activations halved host-side.
    # WhB is halved too: WhB^T rm = (WhB/2)^T (r' me) + (WhB/2)^T me.
    shared = dict(
        emb=w(emb),
        WzT=w(0.5 * Wz[:H]), WzB=w(0.5 * Wz[H:]),
        WhT=w(Wh[:H]), WhB=w(0.5 * Wh[H:]),
        Wr=w(0.5 * Wr_), Ur=w(0.5 * Ur_),
        UwX=w(U_w[:H]), UwH=w(U_w[H : 2 * H]), UwL=w(U_w[2 * H :]),
        WwH=w(W_w[:H]), WwL=w(W_w[H:]),
        Wo=w(Wo_w), Us=w(Us_w),
        bz2=c2(0.5 * bz), bh2=c2(bh), br2=c2(0.5 * br),
        ub2=c2(U_b), wb2=c2(W_b),
        usb=np.full((128, 1), float(Us_b.reshape(-1)[0]), np.float32),
    )
    wob_nonzero = bool(np.any(Wo_b != 0))
    if wob_nonzero:
        shared["wob"] = Wo_b.reshape(1, V)

    ii, pprt = np.meshgrid(np.arange(48), np.arange(128), indexing="xy")
    tblk = 2 * ii + pprt // 64
    ptgt = np.ascontiguousarray((tblk <= 46).astype(np.float32))

    in_maps = []
    for c in range(NC):
        w2 = wid[c * BC : (c + 1) * BC]
        flat = np.ascontiguousarray(w2.T).reshape(-1)
        m = dict(shared)
        m["gidx"] = np.ascontiguousarray(flat.reshape(24, 128)).astype(np.int32)
        m["tvt"] = np.ascontiguousarray(
            np.tile(tree_vec[c * BC : (c + 1) * BC].T, (1, 8))
        ).astype(wdt)
        m["qtgt"] = np.ascontiguousarray(flat.reshape(24, 128).T).astype(np.float32)
        m["ptgt"] = ptgt
        in_maps.append(m)
    return in_maps, wob_nonzero, float(Us_b.reshape(-1)[0])


def _combine(results, us_b):
    S = np.zeros(8, np.float64)
    for r in results:
        S += np.asarray(r["outp"], np.float64).sum(axis=0)
    pad_bce = max(us_b, 0.0) + np.log1p(np.exp(-abs(us_b)))
    pad_corr = 1.0 if us_b <= 0 else 0.0
    n_pad = NC * (PPAD - PROWS)
    p_loss = (S[0] - n_pad * pad_bce) / B
    p_acc = (S[1] - n_pad * pad_corr) / (PBLK * B)
    q_loss = (S[2] - S[3]) / B
    q_acc = S[4] / (QBLK * B)
    return np.array([q_loss, p_loss, q_acc, p_acc], np.float32)


def kernel(**inputs) -> np.ndarray:
    from concourse.bass_utils import run_bass_kernel_spmd

    in_maps, wob_nonzero, us_b = _prep_inputs(inputs)
    nc = _get_nc(wob_nonzero)
    res = run_bass_kernel_spmd(nc, in_maps, list(range(NC)))
    return _combine(res.results, us_b)


# revision 68
# speedup vs baseline: 1.5205x; 1.0310x over previous
"""DGLJTNNDecoder kernel for 8x Trainium2 NeuronCores (Bass/Tile), v3.

Tree-GRU decoder over B=512 chain-trees (N=48 nodes), T=94 DFS steps,
followed by two MLP heads producing (q_loss, p_loss, q_acc, p_acc).
Data-parallel over trees, 64 trees per core; host combines partials.

v3 highlights:
  - sigmoids as scaled tanh (sigma(x) = (1+tanh(x/2))/2, halving folded
    into host-prescaled Wz/bz/Wr/Ur/br) so everything up to the tail
    runs out of the exp_and_others ACT table set (tanh/exp/relu/
    identity/abs); a single table switch at the tail covers the Ln ops.
  - the fwd and bwd DFS chains share one interleaved history tile
    hist[128, kt, slot, chain, 64]: slot t holds exactly the two state
    vectors iteration t consumes, so every state matmul runs at N=128
    with one LDWEIGHTS per weight tile, and all scan elementwise/tanh
    ops are fused across chains (FD=256).
  - A_z and A_r live interleaved in one arena so a single N=256
    identity-matmul per chain prefills the z+r PSUM bank (biases are
    folded into the projections in phase B).
  - h_v is accumulated in node order: hbw[n] = m_bwd[slot 47-n] +
    m_fwd[slot n]  (also fixes v1's off-by-one).
  - gathers/weight DMAs fan out over four trigger queues and the
    embedding transposes chase them, so the prefix is short; head
    matmuls (phase-B remainder, p0/p1/q1/q2/p2) are pumped between
    scan iterations with psum evacuations deferred one iteration.
"""

import sys

if "/opt/trn_rl_repo" not in sys.path:
    sys.path.insert(0, "/opt/trn_rl_repo")

import numpy as np

B, N, H, L, V = 512, 48, 256, 64, 800
NC = 8
BC = B // NC            # 64 trees per core
NF = N - 1              # 47 forward steps (= backward steps)
T = 2 * NF              # 94
NODES = N * BC          # 3072 gathered node rows per core
QBLK = NF + 1           # 48 q-head blocks
PBLK = T + 1            # 95 p-head blocks
PROWS = PBLK * BC       # 6080
PPAD = 48 * 128         # 6144

BF16 = True

_CACHE = {}


def _build(wob_nonzero: bool):
    import concourse.bass as bass
    import concourse.tile as tile
    from concourse import bacc, mybir
    from concourse.masks import make_identity

    f32 = mybir.dt.float32
    i32 = mybir.dt.int32
    wdt = mybir.dt.bfloat16 if BF16 else f32
    AF = mybir.ActivationFunctionType
    ALU = mybir.AluOpType
    AX = mybir.AxisListType

    nc = bacc.Bacc()

    def din(name, shape, dtype=f32):
        return nc.declare_dram_parameter(name, list(shape), dtype, isOutput=False)

    gidx = din("gidx", [24, 128], i32)
    tvt = din("tvt", [L, 8 * BC], wdt)
    qtgt = din("qtgt", [128, 24])
    ptgt = din("ptgt", [128, 48])
    emb = din("emb", [V, H], wdt)
    WzT = din("WzT", [H, H], wdt); WzB = din("WzB", [H, H], wdt)
    WhT = din("WhT", [H, H], wdt); WhB = din("WhB", [H, H], wdt)
    Wr = din("Wr", [H, H], wdt); Ur = din("Ur", [H, H], wdt)
    UwX = din("UwX", [H, H], wdt); UwH = din("UwH", [H, H], wdt)
    UwL = din("UwL", [L, H], wdt)
    WwH = din("WwH", [H, H], wdt); WwL = din("WwL", [L, H], wdt)
    Wo = din("Wo", [H, V], wdt); Us = din("Us", [H, 1], wdt)
    bz2 = din("bz2", [128, 2]); bh2 = din("bh2", [128, 2]); br2 = din("br2", [128, 2])
    ub2 = din("ub2", [128, 2]); wb2 = din("wb2", [128, 2])
    usb = din("usb", [128, 1])
    wob = din("wob", [1, V]) if wob_nonzero else None
    outp = nc.declare_dram_parameter("outp", [128, 8], f32, isOutput=True)

    def rearr2(ap):
        return ap.rearrange("(k p) m -> p k m", p=128)

    with tile.TileContext(nc) as tc:
        with (
            tc.tile_pool(name="persist", bufs=1) as pp,
            tc.tile_pool(name="small", bufs=1) as sp,
        ):
            # --- DMA triggers fan out over queues (gpsimd reserved for
            # the indirect gathers, which only it can issue) --------------
            dma_queues = [nc.sync, nc.scalar]
            dq_i = [0]

            def dma_next(out, in_):
                eng = dma_queues[dq_i[0] % len(dma_queues)]
                dq_i[0] += 1
                eng.dma_start(out=out, in_=in_)

            idx_s = pp.tile([128, 24], i32, tag="idx")
            nc.sync.dma_start(out=idx_s, in_=gidx[:].rearrange("c p -> p c"))

            def loadw(dram, shape, tag, dt=wdt, re2=True):
                t = pp.tile(shape, dt, tag=tag)
                dma_next(t, rearr2(dram[:]) if re2 else dram[:])
                return t

            wzb_s = loadw(WzB, [128, 2, H], "wzb")
            whb_s = loadw(WhB, [128, 2, H], "whb")
            ur_s = loadw(Ur, [128, 2, H], "ur")
            wzt_s = loadw(WzT, [128, 2, H], "wzt")
            wht_s = loadw(WhT, [128, 2, H], "wht")
            wr_s = loadw(Wr, [128, 2, H], "wr")
            uwx_s = loadw(UwX, [128, 2, H], "uwx")
            uwh_s = loadw(UwH, [128, 2, H], "uwh")
            wwh_s = loadw(WwH, [128, 2, H], "wwh")
            wo_s = loadw(Wo, [128, 2, V], "wo")
            us_s = loadw(Us, [128, 2, 1], "us")
            uwl_s = loadw(UwL, [L, H], "uwl", re2=False)
            wwl_s = loadw(WwL, [L, H], "wwl", re2=False)
            bz_s = loadw(bz2, [128, 2], "bz", dt=f32, re2=False)
            bh_s = loadw(bh2, [128, 2], "bh", dt=f32, re2=False)
            br_s = loadw(br2, [128, 2], "br", dt=f32, re2=False)
            ub_s = loadw(ub2, [128, 2], "ub", dt=f32, re2=False)
            wb_s = loadw(wb2, [128, 2], "wb", dt=f32, re2=False)
            usb_s = loadw(usb, [128, 1], "usb", dt=f32, re2=False)
            qtgt_s = loadw(qtgt, [128, 24], "qtgt", dt=f32, re2=False)
            ptgt_s = loadw(ptgt, [128, 48], "ptgt", dt=f32, re2=False)
            wob_s = loadw(wob, [1, V], "wob", dt=f32, re2=False) if wob_nonzero else None

            tvrep = pp.tile([L, 8, BC], wdt, tag="tvrep")
            dma_next(tvrep, tvt[:].rearrange("l (r b) -> l r b", b=BC))

            ident = pp.tile([128, 128], wdt, tag="ident")
            make_identity(nc, ident)

            iota_f = pp.tile([128, V], f32, tag="iotaf")
            iota_i = pp.tile([128, V], i32, tag="iotai")
            nc.gpsimd.iota(iota_i, pattern=[[1, V]], base=0, channel_multiplier=0)
            nc.vector.tensor_copy(iota_f, iota_i)

            # persistent tensors
            xt = pp.tile([128, 2, NODES], wdt, tag="xt")
            hist = pp.tile([128, 2, QBLK, 2, BC], wdt, tag="hist")  # [kt,slot,ch,b]
            hbw = pp.tile([128, 2, QBLK, BC], wdt, tag="hbw")       # node order
            azr = pp.tile([128, 2, 2, NODES], wdt, tag="azr")       # [gate(z,r),mt,*]
            ah = pp.tile([128, 2, NODES], wdt, tag="ah")
            p0a = pp.tile([128, 2, NODES], wdt, tag="p0a")
            p1a = pp.tile([128, 2, NODES], wdt, tag="p1a")
            q1a = pp.tile([128, 2, NODES], wdt, tag="q1a")
            scr = pp.tile([128, 1024], f32, tag="scr")
            scr2 = pp.tile([128, 1024], f32, tag="scr2")

            nc.vector.memset(hist[:, :, 0, :, :], 0.0)
            nc.vector.memset(p1a[:, :, NF * BC:], 0.0)

            outp_s = sp.tile([128, 8], f32, tag="outp")
            nc.vector.memset(outp_s, 0.0)
            sume_acc = sp.tile([128, 48], f32, tag="sume")
            qt_acc = sp.tile([128, 48], f32, tag="qta")
            rmax_acc = sp.tile([128, 48], f32, tag="rmx")

            hbw_f = hbw.rearrange("p k s b -> p k (s b)")
            xt_v = xt.rearrange("p k (c f) -> p k c f", f=128)

            # mfq-equivalent strided views of hist (fwd chain, ch=0):
            def mfq_cols(kt, s0, ns):
                return hist[:, kt, s0 : s0 + ns, 0, :]

            # ================= scan-era pools ==========================
            # PSUM (8 banks): tps x1 + zr x2 + h x1 + head-pipe x4.
            with (
                tc.tile_pool(name="gath", bufs=1) as gp,
                tc.tile_pool(name="tps", bufs=1, space="PSUM") as tpp,
                tc.tile_pool(name="zrp", bufs=2, space="PSUM") as zr_p,
                tc.tile_pool(name="hp", bufs=1, space="PSUM") as h_p,
                tc.tile_pool(name="hps", bufs=4, space="PSUM") as hps_p,
                tc.tile_pool(name="st", bufs=4) as st,
            ):
                # --- Phase A: all 24 gathers issue back-to-back on the
                # gpsimd queue (distinct buffers, no tag reuse); the
                # transposes chase them - the first four pairs feed the
                # prefix phase-B chunks, the rest are pumped as era units.
                gather_order = [20, 21, 22, 23, 0, 1, 2, 3,
                                16, 17, 18, 19, 4, 5, 6, 7,
                                12, 13, 14, 15, 8, 9, 10, 11]
                xg_tiles = {}
                for c in gather_order:
                    xg = gp.tile([128, H], wdt, tag=f"xg{c}")
                    nc.gpsimd.indirect_dma_start(
                        out=xg, out_offset=None, in_=emb[:],
                        in_offset=bass.IndirectOffsetOnAxis(
                            ap=idx_s[:, c : c + 1], axis=0))
                    xg_tiles[c] = xg

                evac_seq = [0]

                def transpose_pair(c0):
                    pt = tpp.tile([128, 2, 2, 128], wdt, tag="pt")
                    for i, c in enumerate((c0, c0 + 1)):
                        for ht in range(2):
                            nc.tensor.transpose(
                                pt[:, ht, i, :],
                                xg_tiles[c][:, ht * 128 : (ht + 1) * 128], ident)

                    def evac():
                        evac_seq[0] ^= 1
                        if evac_seq[0]:
                            nc.scalar.copy(xt_v[:, :, c0 : c0 + 2, :], pt)
                        else:
                            nc.vector.tensor_copy(xt_v[:, :, c0 : c0 + 2, :], pt)
                    return evac

                # warm the PE clock (HAM) while the first gathers land:
                # harmless matmuls on the identity into a scratch bank.
                warm = tpp.tile([128, 512], f32, tag="pt")
                for i in range(24):
                    nc.tensor.matmul(
                        warm[:, :128], ident, ident,
                        start=(i == 0), stop=(i == 23))

                for c0 in (20, 22, 0, 2):
                    transpose_pair(c0)()

                # --- evac engine alternation (2:1 toward ACT; DVE is the
                # hotter engine during the era) --------------------------
                eng_flip = [0]

                def _evac_on_act():
                    eng_flip[0] = (eng_flip[0] + 1) % 3
                    return eng_flip[0] != 0

                def evac_relu(dst, ps, bias_ap):
                    if _evac_on_act():
                        nc.scalar.activation(dst, ps, AF.Relu, bias=bias_ap)
                    else:
                        nc.vector.tensor_scalar(
                            out=dst, in0=ps, scalar1=bias_ap, scalar2=0.0,
                            op0=ALU.add, op1=ALU.max)

                def evac_ident(dst, ps, bias_ap):
                    if _evac_on_act():
                        nc.scalar.activation(dst, ps, AF.Identity, bias=bias_ap)
                    else:
                        nc.vector.tensor_scalar(
                            out=dst, in0=ps, scalar1=bias_ap, scalar2=None,
                            op0=ALU.add)

                # --- units ---------------------------------------------
                def b_unit(mat, ch, mt):
                    w_s, b_s = {
                        "z": (wzt_s, bz_s), "h": (wht_s, bh_s),
                        "r": (wr_s, br_s)}[mat]
                    dst = {"z": azr[:, 0], "h": ah, "r": azr[:, 1]}[mat]
                    msl = slice(mt * 128, (mt + 1) * 128)
                    csl = slice(ch * 512, (ch + 1) * 512)
                    ps = hps_p.tile([128, 512], f32, tag="hps")
                    for kt in range(2):
                        nc.tensor.matmul(
                            ps, w_s[:, kt, msl], xt[:, kt, csl],
                            start=(kt == 0), stop=(kt == 1))
                    return lambda: evac_ident(dst[:, mt, csl], ps, b_s[:, mt:mt+1])

                def p0_unit(ch, mt):
                    msl = slice(mt * 128, (mt + 1) * 128)
                    csl = slice(ch * 512, (ch + 1) * 512)
                    ps = hps_p.tile([128, 512], f32, tag="hps")
                    for kt in range(2):
                        nc.tensor.matmul(
                            ps, uwx_s[:, kt, msl], xt[:, kt, csl],
                            start=(kt == 0), stop=False)
                    for kt in range(2):
                        nc.tensor.matmul(
                            ps, uwh_s[:, kt, msl], mfq_cols(kt, 8 * ch, 8),
                            start=False, stop=False)
                    nc.tensor.matmul(
                        ps, uwl_s[:, msl], tvrep[:, :8, :],
                        start=False, stop=True)
                    return lambda: evac_relu(p0a[:, mt, csl], ps, ub_s[:, mt:mt+1])

                def p1_unit(u, mt):
                    n0 = 4 * u
                    nn = min(4, NF - n0)
                    cw = nn * BC
                    msl = slice(mt * 128, (mt + 1) * 128)
                    csl = slice(n0 * BC, n0 * BC + cw)
                    ps = hps_p.tile([128, 512], f32, tag="hps")
                    psv = ps[:, :cw]
                    for kt in range(2):
                        nc.tensor.matmul(
                            psv, uwx_s[:, kt, msl], xt[:, kt, csl],
                            start=(kt == 0), stop=False)
                    for kt in range(2):
                        nc.tensor.matmul(
                            psv, uwh_s[:, kt, msl], hbw_f[:, kt, csl],
                            start=False, stop=False)
                    nc.tensor.matmul(
                        psv, uwl_s[:, msl], tvrep[:, :nn, :],
                        start=False, stop=True)
                    return lambda: evac_relu(p1a[:, mt, csl], psv, ub_s[:, mt:mt+1])

                def q1_unit(ch, mt):
                    msl = slice(mt * 128, (mt + 1) * 128)
                    csl = slice(ch * 512, (ch + 1) * 512)
                    ps = hps_p.tile([128, 512], f32, tag="hps")
                    for kt in range(2):
                        nc.tensor.matmul(
                            ps, wwh_s[:, kt, msl], mfq_cols(kt, 8 * ch, 8),
                            start=(kt == 0), stop=False)
                    nc.tensor.matmul(
                        ps, wwl_s[:, msl], tvrep[:, :8, :],
                        start=False, stop=True)
                    return lambda: evac_relu(q1a[:, mt, csl], ps, wb_s[:, mt:mt+1])

                def q2_unit(j, half):
                    # vocab half of one logits row-tile through the shared
                    # head-pipe pool; softmax partials (sume/qt/rmax) are
                    # accumulated per half and combined in three tail ops.
                    n0, nn = (0, 512) if half == 0 else (512, V - 512)
                    col = half * 24 + j
                    psq = hps_p.tile([128, 512], f32, tag="hps")
                    pv = psq[:, :nn]
                    jsl = slice(j * 128, (j + 1) * 128)
                    for kt in range(2):
                        nc.tensor.matmul(
                            pv, q1a[:, kt, jsl], wo_s[:, kt, n0 : n0 + nn],
                            start=(kt == 0), stop=(kt == 1))
                    if wob_nonzero:
                        wv = wob_s[:]
                        wb_b = bass.AP(
                            tensor=wv.tensor, offset=wv.offset + n0,
                            ap=[[0, 128], [1, nn]])
                        nc.vector.tensor_add(pv, pv, wb_b)

                    def softmax():
                        nc.scalar.activation(
                            scr[:, :nn], pv, AF.Exp,
                            accum_out=sume_acc[:, col : col + 1])
                        nc.vector.scalar_tensor_tensor(
                            out=scr2[:, :nn], in0=iota_f[:, n0 : n0 + nn],
                            scalar=qtgt_s[:, j : j + 1],
                            in1=pv, op0=ALU.is_equal, op1=ALU.mult,
                            accum_out=qt_acc[:, col : col + 1])
                        nc.vector.reduce_max(
                            rmax_acc[:, col : col + 1], pv, axis=AX.X)
                    return softmax

                # p2 sub-units: a few p1-row tiles reduced against Us as
                # soon as their relu arena columns are complete, straight
                # into the matching p_sb slice (no persistent psum).
                p_sb = sp.tile([128, 48], f32, tag="psb")

                def p2_unit(src, j0, nj, col0):
                    ps = hps_p.tile([128, 512], f32, tag="hps")
                    for j in range(j0, j0 + nj):
                        for kt in range(2):
                            nc.tensor.matmul(
                                ps[:, j - j0 : j - j0 + 1],
                                src[:, kt, j * 128 : (j + 1) * 128],
                                us_s[:, kt, :],
                                start=(kt == 0), stop=(kt == 1))
                    return lambda: nc.scalar.activation(
                        p_sb[:, col0 : col0 + nj], ps[:, :nj],
                        AF.Identity, bias=usb_s[:, 0:1])

                # --- pump scheduler ------------------------------------
                pending = []
                seq_ctr = [0]

                def enq(ready, kind, fn, deadline=10**9):
                    pending.append([ready, seq_ctr[0], kind, deadline, fn])
                    seq_ctr[0] += 1

                evacs_next = []
                q1_emitted = {}
                p0_emitted = {}
                p1_emitted = {}
                cur_t = [0]

                def flush_evacs():
                    for ev in evacs_next:
                        ev()
                    evacs_next.clear()

                def pump(t, nh=3):
                    flush_evacs()
                    budget = {"hps": nh, "tps": 1}
                    pending.sort(key=lambda u: (u[0], u[1]))
                    for u in list(pending):
                        ready, _, kind, deadline, fn = u
                        if ready > t or budget[kind] == 0:
                            continue
                        assert t <= deadline, f"unit past deadline at iter {t}"
                        budget[kind] -= 1
                        pending.remove(u)
                        evacs_next.append(fn())

                # phase-B chunks 5,0 then 4,1 in the prefix (the scan
                # reads them by iters 0 and 8); software-pipelined evacs
                # (hps bufs=2 -> <=2 in flight).
                def b_prefix(chunks):
                    prev_ev = None
                    for ch in chunks:
                        for mat in ("z", "r", "h"):
                            for mt in range(2):
                                ev = b_unit(mat, ch, mt)
                                if prev_ev is not None:
                                    prev_ev()
                                prev_ev = ev
                    prev_ev()

                b_prefix((5, 0))
                # all remaining transposes and B chunks are era units so
                # the scan starts right after B(5,0): pair at pump p
                # writes xt at pump p+1 (evacs run first); B chunk ready
                # gates on its transposes' evacs, deadline = first-read
                # iteration minus 2 (unit at pump p -> azr written at
                # pump p+1, before scan_iter(p+2)'s prefill).
                for c0, p in ((16, 0), (18, 1), (4, 2), (6, 3),
                              (12, 4), (14, 5), (8, 6), (10, 7)):
                    enq(p, "tps", lambda cc=c0: transpose_pair(cc), deadline=p)
                for ch, rdy, dl in ((4, 2, 8), (1, 4, 8), (3, 6, 16), (2, 8, 16)):
                    for mat in ("z", "r", "h"):
                        for mt in range(2):
                            enq(rdy, "hps",
                                lambda m=mat, c=ch, k=mt: b_unit(m, c, k),
                                deadline=dl - 2)

                def _after_p0(c, k):
                    ev = p0_unit(c, k)
                    def done():
                        ev()
                        p0_emitted[c] = p0_emitted.get(c, 0) + 1
                        if p0_emitted[c] == 2:
                            # p0a chunk c complete -> p-row tiles 4c..4c+3
                            enq(cur_t[0] + 1, "hps",
                                lambda cc=c: p2_unit(p0a, 4 * cc, 4, 4 * cc))
                    return done

                def _after_p1(u, k):
                    ev = p1_unit(u, k)
                    def done():
                        ev()
                        p1_emitted[u] = p1_emitted.get(u, 0) + 1
                        if p1_emitted[u] == 2:
                            # p1a nodes 4u..4u+3 complete -> tiles 2u,2u+1
                            enq(cur_t[0] + 1, "hps",
                                lambda uu=u: p2_unit(p1a, 2 * uu, 2, 24 + 2 * uu))
                    return done

                for ch in range(6):
                    rdy = min(8 * ch + 7, NF)
                    for mt in range(2):
                        enq(rdy, "hps", lambda c=ch, k=mt: _after_p0(c, k))

                        def _q1(c=ch, k=mt):
                            ev = q1_unit(c, k)
                            q1_emitted[c] = q1_emitted.get(c, 0) + 1
                            if q1_emitted[c] == 2:
                                for j in range(4 * c, 4 * c + 4):
                                    for hf in range(2):
                                        enq(cur_t[0] + 1, "hps",
                                            lambda jj=j, h=hf: q2_unit(jj, h))
                            return ev
                        enq(rdy, "hps", _q1)
                for u in range(12):
                    n0 = 4 * u
                    nn = min(4, NF - n0)
                    rdy = max(max(NF - 1 - n, n - 1) for n in range(n0, n0 + nn)) + 1
                    for mt in range(2):
                        enq(rdy, "hps", lambda uu=u, k=mt: _after_p1(uu, k))

                # --- scan state tiles (fused across chains) ------------
                zr_sb = pp.tile([128, 2, 2, 2, BC], wdt, tag="zrsb")  # [gate,mt,ch,b]
                rm_fb = pp.tile([128, 2, 2, BC], wdt, tag="rmfb")

                def zr_prefill(t):
                    # A_z/A_r pulled into the zr bank; independent of the
                    # scan state, so it is emitted at the END of iteration
                    # t-1 (before the pumped head units) and runs in the
                    # me-combine window on the other zr buffer.
                    s_f, s_b = t, NF - t
                    ps = zr_p.tile([128, 2, 2, 2, BC], f32, tag="zr")
                    nc.tensor.matmul(
                        ps[:, :, :, 0, :], ident,
                        azr[:, :, :, s_f * BC : (s_f + 1) * BC],
                        start=True, stop=False)
                    nc.tensor.matmul(
                        ps[:, :, :, 1, :], ident,
                        azr[:, :, :, s_b * BC : (s_b + 1) * BC],
                        start=False, stop=False)
                    return ps

                def scan_iter(t, ps):
                    """Interleaved fwd+bwd GRU iteration t (fused chains).

                    src_f = t, src_b = 47-t; dst(t-1) == src(t) on a chain.
                    psum zr: [gate2, mt2, ch2, 64]; psum h: [mt2, ch2, 64].
                    Critical path: me(t-1) -> r mms -> tanh_r -> rm' ->
                    h rm-mms -> tanh_h -> zpmt -> me(t); everything else
                    (z gate, h me-part mms, ozs) runs off that path.
                    rm = (1+r')/2 * me is folded into the h matmuls:
                    WhB^T rm = WhB2^T(r' me) + WhB2^T me  (WhB2 = WhB/2,
                    prescaled host-side).
                    """
                    s_f, s_b = t, NF - t
                    s_ap = hist[:, :, t, :, :]
                    # r-gate state matmuls first: tanh_r is on the critical
                    # path, z is not.
                    gates = [(1, ur_s), (0, wzb_s)] if t > 0 else [(0, wzb_s)]
                    for gi, (g, w_s) in enumerate(gates):
                        for mt in range(2):
                            msl = slice(mt * 128, (mt + 1) * 128)
                            for kt in range(2):
                                nc.tensor.matmul(
                                    ps[:, g, mt, :, :], w_s[:, kt, msl],
                                    s_ap[:, kt, :, :],
                                    start=False,
                                    stop=(gi == len(gates) - 1 and mt == 1
                                          and kt == 1))
                    if t > 0:
                        nc.scalar.activation(zr_sb[:, 1], ps[:, 1], AF.Tanh)
                    # H bank: A_h prefill + the state me-part (off-path)
                    ps_h = h_p.tile([128, 2, 2, BC], f32, tag="hh")
                    nc.tensor.matmul(
                        ps_h[:, :, 0, :], ident,
                        ah[:, :, s_f * BC : (s_f + 1) * BC],
                        start=True, stop=False)
                    nc.tensor.matmul(
                        ps_h[:, :, 1, :], ident,
                        ah[:, :, s_b * BC : (s_b + 1) * BC],
                        start=False, stop=(t == 0))
                    if t > 0:
                        for mt in range(2):
                            msl = slice(mt * 128, (mt + 1) * 128)
                            for kt in range(2):
                                nc.tensor.matmul(
                                    ps_h[:, mt, :, :], whb_s[:, kt, msl],
                                    s_ap[:, kt, :, :],
                                    start=False, stop=False)
                    nc.scalar.activation(zr_sb[:, 0], ps[:, 0], AF.Tanh)
                    if t > 0:
                        # rm' = r' * me(t-1); h rm-part matmuls (on path);
                        # WhB^T rm = WhB2^T (r' me) + WhB2^T me, WhB2
                        # prescaled on the host.
                        nc.vector.tensor_mul(rm_fb, zr_sb[:, 1], s_ap)
                        for mt in range(2):
                            msl = slice(mt * 128, (mt + 1) * 128)
                            for kt in range(2):
                                nc.tensor.matmul(
                                    ps_h[:, mt, :, :], whb_s[:, kt, msl],
                                    rm_fb[:, kt, :, :],
                                    start=False, stop=(mt == 1 and kt == 1))

                    # off-path: zp = 0.5 z' + 0.5 ; ozs = (0.5 - 0.5 z') s
                    zp = st.tile([128, 2, 2, BC], wdt, tag="zp")
                    nc.vector.tensor_scalar(
                        out=zp, in0=zr_sb[:, 0], scalar1=0.5, scalar2=0.5,
                        op0=ALU.mult, op1=ALU.add)
                    on = st.tile([128, 2, 2, BC], wdt, tag="on")
                    nc.vector.tensor_scalar(
                        out=on, in0=zr_sb[:, 0], scalar1=-0.5, scalar2=0.5,
                        op0=ALU.mult, op1=ALU.add)
                    ozs = st.tile([128, 2, 2, BC], wdt, tag="ozs")
                    nc.vector.tensor_mul(ozs, on, s_ap)
                    # on path: mt = tanh(psum_h); me = zp*mt + ozs
                    mt_sb = st.tile([128, 2, 2, BC], wdt, tag="mts")
                    nc.scalar.activation(mt_sb, ps_h, AF.Tanh)
                    zpmt = st.tile([128, 2, 2, BC], wdt, tag="zpmt")
                    nc.vector.tensor_mul(zpmt, zp, mt_sb)
                    nc.vector.tensor_add(hist[:, :, t + 1, :, :], zpmt, ozs)
                    # h_v adds ready at t: hbw[n] = m_b[slot 47-n] + m_f[slot n]
                    for n in range(NF):
                        if max(NF - 1 - n, n - 1) == t:
                            nc.vector.tensor_add(
                                hbw[:, :, n, :],
                                hist[:, :, QBLK - 1 - n, 1, :],
                                hist[:, :, n, 0, :])

                for t in range(NF):
                    cur_t[0] = t
                    scan_iter(t, zr_prefill(t))
                    pump(t, nh=3)
                t = NF
                while pending or evacs_next:
                    cur_t[0] = t
                    pump(t, nh=3)
                    t += 1

                # --- tail: combine vocab-half softmax partials, BCE ----
                sume_c = sp.tile([128, 24], f32, tag="sumc")
                nc.vector.tensor_add(sume_c, sume_acc[:, :24], sume_acc[:, 24:])
                qt_c = sp.tile([128, 24], f32, tag="qtc")
                nc.vector.tensor_add(qt_c, qt_acc[:, :24], qt_acc[:, 24:])
                rmax_c = sp.tile([128, 24], f32, tag="rmc")
                nc.vector.tensor_tensor(
                    out=rmax_c, in0=rmax_acc[:, :24], in1=rmax_acc[:, 24:],
                    op=ALU.max)
                qc_t = sp.tile([128, 24], f32, tag="qct")
                nc.vector.tensor_tensor(
                    out=qc_t, in0=qt_c, in1=rmax_c, op=ALU.is_ge)
                ab_t = sp.tile([128, 48], f32, tag="abt")
                nc.scalar.activation(ab_t, p_sb, AF.Abs)
                en_t = sp.tile([128, 48], f32, tag="ent")
                nc.scalar.activation(en_t, ab_t, AF.Exp, scale=-1.0)
                rl_t = sp.tile([128, 48], f32, tag="rlt")
                nc.scalar.activation(rl_t, p_sb, AF.Relu)
                l1p_t = sp.tile([128, 48], f32, tag="l1p")
                nc.scalar.activation(l1p_t, en_t, AF.Ln, bias=1.0)
                lse_t = sp.tile([128, 24], f32, tag="lse")
                nc.scalar.activation(lse_t, sume_c, AF.Ln)

                sp_t = sp.tile([128, 48], f32, tag="spt")
                nc.vector.tensor_add(sp_t, l1p_t, rl_t)
                ptt = sp.tile([128, 48], f32, tag="ptt")
                nc.vector.tensor_mul(ptt, p_sb, ptgt_s)
                bce = sp.tile([128, 48], f32, tag="bce")
                nc.vector.tensor_sub(bce, sp_t, ptt)
                nc.vector.reduce_sum(outp_s[:, 0:1], bce, axis=AX.X)
                gtz = sp.tile([128, 48], f32, tag="gtz")
                nc.vector.tensor_scalar(
                    out=gtz, in0=p_sb, scalar1=0.0, scalar2=None, op0=ALU.is_gt)
                pcr = sp.tile([128, 48], f32, tag="pcr")
                nc.vector.tensor_tensor(out=pcr, in0=gtz, in1=ptgt_s,
                                        op=ALU.is_equal)
                nc.vector.reduce_sum(outp_s[:, 1:2], pcr, axis=AX.X)
                nc.vector.reduce_sum(outp_s[:, 2:3], lse_t, axis=AX.X)
                nc.vector.reduce_sum(outp_s[:, 3:4], qt_c, axis=AX.X)
                nc.vector.reduce_sum(outp_s[:, 4:5], qc_t, axis=AX.X)
            nc.sync.dma_start(out=outp[:], in_=outp_s)

    nc.finalize()
    return nc


def _get_nc(wob_nonzero: bool):
    key = ("nc", wob_nonzero, BF16)
    if key not in _CACHE:
        _CACHE[key] = _build(wob_nonzero)
    return _CACHE[key]


def _wdt_np():
    if BF16:
        import ml_dtypes

        return ml_dtypes.bfloat16
    return np.float32


def _prep_inputs(inputs):
    f = lambda k: np.ascontiguousarray(np.asarray(inputs[k]), dtype=np.float32)
    wdt = _wdt_np()
    w = lambda a: np.ascontiguousarray(a).astype(wdt)
    wid = np.asarray(inputs["wid"]).astype(np.int64).reshape(B, N)
    tree_vec = f("tree_vec")
    Wz, bz = f("Wz"), f("bz")
    Wr_, Ur_, br = f("Wr"), f("Ur"), f("br")
    Wh, bh = f("Wh"), f("bh")
    W_w, W_b = f("W_w"), f("W_b")
    U_w, U_b = f("U_w"), f("U_b")
    Wo_w, Wo_b = f("Wo_w"), f("Wo_b")
    Us_w, Us_b = f("Us_w"), f("Us_b")
    emb = f("embedding")

    def c2(v):
        return np.ascontiguousarray(v.reshape(2, 128).T)

    # sigma(x) = (1+tanh(x/2))/2: z and r pre-activations halved host-side.
    # WhB is halved too: WhB^T rm = (WhB/2)^T (r' me) + (WhB/2)^T me.
    shared = dict(
        emb=w(emb),
        WzT=w(0.5 * Wz[:H]), WzB=w(0.5 * Wz[H:]),
        WhT=w(Wh[:H]), WhB=w(0.5 * Wh[H:]),
        Wr=w(0.5 * Wr_), Ur=w(0.5 * Ur_),
        UwX=w(U_w[:H]), UwH=w(U_w[H : 2 * H]), UwL=w(U_w[2 * H :]),
        WwH=w(W_w[:H]), WwL=w(W_w[H:]),
        Wo=w(Wo_w), Us=w(Us_w),
        bz2=c2(0.5 * bz), bh2=c2(bh), br2=c2(0.5 * br),
        ub2=c2(U_b), wb2=c2(W_b),
        usb=np.full((128, 1), float(Us_b.reshape(-1)[0]), np.float32),
    )
    wob_nonzero = bool(np.any(Wo_b != 0))
    if wob_nonzero:
        shared["wob"] = Wo_b.reshape(1, V)

    ii, pprt = np.meshgrid(np.arange(48), np.arange(128), indexing="xy")
    tblk = 2 * ii + pprt // 64
    ptgt = np.ascontiguousarray((tblk <= 46).astype(np.float32))

    in_maps = []
    for c in range(NC):
        w2 = wid[c * BC : (c + 1) * BC]
        flat = np.ascontiguousarray(w2.T).reshape(-1)
        m = dict(shared)
        m["gidx"] = np.ascontiguousarray(flat.reshape(24, 128)).astype(np.int32)
        m["tvt"] = np.ascontiguousarray(
            np.tile(tree_vec[c * BC : (c + 1) * BC].T, (1, 8))
        ).astype(wdt)
        m["qtgt"] = np.ascontiguousarray(flat.reshape(24, 128).T).astype(np.float32)
        m["ptgt"] = ptgt
        in_maps.append(m)
    return in_maps, wob_nonzero, float(Us_b.reshape(-1)[0])


def _combine(results, us_b):
    S = np.zeros(8, np.float64)
    for r in results:
        S += np.asarray(r["outp"], np.float64).sum(axis=0)
    pad_bce = max(us_b, 0.0) + np.log1p(np.exp(-abs(us_b)))
    pad_corr = 1.0 if us_b <= 0 else 0.0
    n_pad = NC * (PPAD - PROWS)
    p_loss = (S[0] - n_pad * pad_bce) / B
    p_acc = (S[1] - n_pad * pad_corr) / (PBLK * B)
    q_loss = (S[2] - S[3]) / B
    q_acc = S[4] / (QBLK * B)
    return np.array([q_loss, p_loss, q_acc, p_acc], np.float32)


def kernel(**inputs) -> np.ndarray:
    from concourse.bass_utils import run_bass_kernel_spmd

    in_maps, wob_nonzero, us_b = _prep_inputs(inputs)
    nc = _get_nc(wob_nonzero)
    res = run_bass_kernel_spmd(nc, in_maps, list(range(NC)))
    return _combine(res.results, us_b)


# revision 71
# speedup vs baseline: 1.5358x; 1.0101x over previous
"""DGLJTNNDecoder kernel for 8x Trainium2 NeuronCores (Bass/Tile), v3.

Tree-GRU decoder over B=512 chain-trees (N=48 nodes), T=94 DFS steps,
followed by two MLP heads producing (q_loss, p_loss, q_acc, p_acc).
Data-parallel over trees, 64 trees per core; host combines partials.

v3 highlights:
  - sigmoids as scaled tanh (sigma(x) = (1+tanh(x/2))/2, halving folded
    into host-prescaled Wz/bz/Wr/Ur/br) so everything up to the tail
    runs out of the exp_and_others ACT table set (tanh/exp/relu/
    identity/abs); a single table switch at the tail covers the Ln ops.
  - the fwd and bwd DFS chains share one interleaved history tile
    hist[128, kt, slot, chain, 64]: slot t holds exactly the two state
    vectors iteration t consumes, so every state matmul runs at N=128
    with one LDWEIGHTS per weight tile, and all scan elementwise/tanh
    ops are fused across chains (FD=256).
  - A_z and A_r live interleaved in one arena so a single N=256
    identity-matmul per chain prefills the z+r PSUM bank (biases are
    folded into the projections in phase B).
  - h_v is accumulated in node order: hbw[n] = m_bwd[slot 47-n] +
    m_fwd[slot n]  (also fixes v1's off-by-one).
  - gathers/weight DMAs fan out over four trigger queues and the
    embedding transposes chase them, so the prefix is short; head
    matmuls (phase-B remainder, p0/p1/q1/q2/p2) are pumped between
    scan iterations with psum evacuations deferred one iteration.
"""

import sys

if "/opt/trn_rl_repo" not in sys.path:
    sys.path.insert(0, "/opt/trn_rl_repo")

import numpy as np

B, N, H, L, V = 512, 48, 256, 64, 800
NC = 8
BC = B // NC            # 64 trees per core
NF = N - 1              # 47 forward steps (= backward steps)
T = 2 * NF              # 94
NODES = N * BC          # 3072 gathered node rows per core
QBLK = NF + 1           # 48 q-head blocks
PBLK = T + 1            # 95 p-head blocks
PROWS = PBLK * BC       # 6080
PPAD = 48 * 128         # 6144

BF16 = True

_CACHE = {}


def _build(wob_nonzero: bool):
    import concourse.bass as bass
    import concourse.tile as tile
    from concourse import bacc, mybir
    from concourse.masks import make_identity

    f32 = mybir.dt.float32
    i32 = mybir.dt.int32
    wdt = mybir.dt.bfloat16 if BF16 else f32
    AF = mybir.ActivationFunctionType
    ALU = mybir.AluOpType
    AX = mybir.AxisListType

    nc = bacc.Bacc()

    def din(name, shape, dtype=f32):
        return nc.declare_dram_parameter(name, list(shape), dtype, isOutput=False)

    gidx = din("gidx", [24, 128], i32)
    tvt = din("tvt", [L, 8 * BC], wdt)
    qtgt = din("qtgt", [128, 24])
    ptgt = din("ptgt", [128, 48])
    emb = din("emb", [V, H], wdt)
    WzT = din("WzT", [H, H], wdt); WzB = din("WzB", [H, H], wdt)
    WhT = din("WhT", [H, H], wdt); WhB = din("WhB", [H, H], wdt)
    Wr = din("Wr", [H, H], wdt); Ur = din("Ur", [H, H], wdt)
    UwX = din("UwX", [H, H], wdt); UwH = din("UwH", [H, H], wdt)
    UwL = din("UwL", [L, H], wdt)
    WwH = din("WwH", [H, H], wdt); WwL = din("WwL", [L, H], wdt)
    Wo = din("Wo", [H, V], wdt); Us = din("Us", [H, 1], wdt)
    bz2 = din("bz2", [128, 2]); bh2 = din("bh2", [128, 2]); br2 = din("br2", [128, 2])
    ub2 = din("ub2", [128, 2]); wb2 = din("wb2", [128, 2])
    usb = din("usb", [128, 1])
    wob = din("wob", [1, V]) if wob_nonzero else None
    outp = nc.declare_dram_parameter("outp", [128, 8], f32, isOutput=True)

    def rearr2(ap):
        return ap.rearrange("(k p) m -> p k m", p=128)

    with tile.TileContext(nc) as tc:
        with (
            tc.tile_pool(name="persist", bufs=1) as pp,
            tc.tile_pool(name="small", bufs=1) as sp,
        ):
            # --- DMA triggers fan out over queues (gpsimd reserved for
            # the indirect gathers, which only it can issue) --------------
            dma_queues = [nc.sync, nc.scalar]
            dq_i = [0]

            def dma_next(out, in_):
                eng = dma_queues[dq_i[0] % len(dma_queues)]
                dq_i[0] += 1
                eng.dma_start(out=out, in_=in_)

            idx_s = pp.tile([128, 24], i32, tag="idx")
            nc.sync.dma_start(out=idx_s, in_=gidx[:].rearrange("c p -> p c"))

            def loadw(dram, shape, tag, dt=wdt, re2=True):
                t = pp.tile(shape, dt, tag=tag)
                dma_next(t, rearr2(dram[:]) if re2 else dram[:])
                return t

            wzb_s = loadw(WzB, [128, 2, H], "wzb")
            whb_s = loadw(WhB, [128, 2, H], "whb")
            ur_s = loadw(Ur, [128, 2, H], "ur")
            wzt_s = loadw(WzT, [128, 2, H], "wzt")
            wht_s = loadw(WhT, [128, 2, H], "wht")
            wr_s = loadw(Wr, [128, 2, H], "wr")
            uwx_s = loadw(UwX, [128, 2, H], "uwx")
            uwh_s = loadw(UwH, [128, 2, H], "uwh")
            wwh_s = loadw(WwH, [128, 2, H], "wwh")
            wo_s = loadw(Wo, [128, 2, V], "wo")
            us_s = loadw(Us, [128, 2, 1], "us")
            uwl_s = loadw(UwL, [L, H], "uwl", re2=False)
            wwl_s = loadw(WwL, [L, H], "wwl", re2=False)
            bz_s = loadw(bz2, [128, 2], "bz", dt=f32, re2=False)
            bh_s = loadw(bh2, [128, 2], "bh", dt=f32, re2=False)
            br_s = loadw(br2, [128, 2], "br", dt=f32, re2=False)
            ub_s = loadw(ub2, [128, 2], "ub", dt=f32, re2=False)
            wb_s = loadw(wb2, [128, 2], "wb", dt=f32, re2=False)
            usb_s = loadw(usb, [128, 1], "usb", dt=f32, re2=False)
            qtgt_s = loadw(qtgt, [128, 24], "qtgt", dt=f32, re2=False)
            ptgt_s = loadw(ptgt, [128, 48], "ptgt", dt=f32, re2=False)
            wob_s = loadw(wob, [1, V], "wob", dt=f32, re2=False) if wob_nonzero else None

            tvrep = pp.tile([L, 8, BC], wdt, tag="tvrep")
            dma_next(tvrep, tvt[:].rearrange("l (r b) -> l r b", b=BC))

            ident = pp.tile([128, 128], wdt, tag="ident")
            make_identity(nc, ident)

            iota_f = pp.tile([128, V], f32, tag="iotaf")
            iota_i = pp.tile([128, V], i32, tag="iotai")
            nc.gpsimd.iota(iota_i, pattern=[[1, V]], base=0, channel_multiplier=0)
            nc.vector.tensor_copy(iota_f, iota_i)

            # persistent tensors
            xt = pp.tile([128, 2, NODES], wdt, tag="xt")
            hist = pp.tile([128, 2, QBLK, 2, BC], wdt, tag="hist")  # [kt,slot,ch,b]
            hbw = pp.tile([128, 2, QBLK, BC], wdt, tag="hbw")       # node order
            azr = pp.tile([128, 2, 2, NODES], wdt, tag="azr")       # [gate(z,r),mt,*]
            ah = pp.tile([128, 2, NODES], wdt, tag="ah")
            p0a = pp.tile([128, 2, NODES], wdt, tag="p0a")
            p1a = pp.tile([128, 2, NODES], wdt, tag="p1a")
            q1a = pp.tile([128, 2, NODES], wdt, tag="q1a")
            scr = pp.tile([128, 1024], f32, tag="scr")
            scr2 = pp.tile([128, 1024], f32, tag="scr2")

            nc.vector.memset(hist[:, :, 0, :, :], 0.0)
            nc.vector.memset(p1a[:, :, NF * BC:], 0.0)

            outp_s = sp.tile([128, 8], f32, tag="outp")
            nc.vector.memset(outp_s, 0.0)
            sume_acc = sp.tile([128, 48], f32, tag="sume")
            qt_acc = sp.tile([128, 48], f32, tag="qta")
            rmax_acc = sp.tile([128, 48], f32, tag="rmx")

            hbw_f = hbw.rearrange("p k s b -> p k (s b)")
            xt_v = xt.rearrange("p k (c f) -> p k c f", f=128)

            # mfq-equivalent strided views of hist (fwd chain, ch=0):
            def mfq_cols(kt, s0, ns):
                return hist[:, kt, s0 : s0 + ns, 0, :]

            # ================= scan-era pools ==========================
            # PSUM (8 banks): tps x1 + zr x2 + h x1 + head-pipe x4.
            with (
                tc.tile_pool(name="gath", bufs=1) as gp,
                tc.tile_pool(name="tps", bufs=1, space="PSUM") as tpp,
                tc.tile_pool(name="zrp", bufs=2, space="PSUM") as zr_p,
                tc.tile_pool(name="hp", bufs=1, space="PSUM") as h_p,
                tc.tile_pool(name="hps", bufs=4, space="PSUM") as hps_p,
                tc.tile_pool(name="st", bufs=4) as st,
            ):
                # --- Phase A: all 24 gathers issue back-to-back on the
                # gpsimd queue (distinct buffers, no tag reuse); the
                # transposes chase them - the first four pairs feed the
                # prefix phase-B chunks, the rest are pumped as era units.
                gather_order = [20, 21, 22, 23, 0, 1, 2, 3,
                                16, 17, 18, 19, 4, 5, 6, 7,
                                12, 13, 14, 15, 8, 9, 10, 11]
                xg_tiles = {}
                for c in gather_order:
                    xg = gp.tile([128, H], wdt, tag=f"xg{c}")
                    nc.gpsimd.indirect_dma_start(
                        out=xg, out_offset=None, in_=emb[:],
                        in_offset=bass.IndirectOffsetOnAxis(
                            ap=idx_s[:, c : c + 1], axis=0))
                    xg_tiles[c] = xg

                evac_seq = [0]

                def transpose_pair(c0):
                    pt = tpp.tile([128, 2, 2, 128], wdt, tag="pt")
                    for i, c in enumerate((c0, c0 + 1)):
                        for ht in range(2):
                            nc.tensor.transpose(
                                pt[:, ht, i, :],
                                xg_tiles[c][:, ht * 128 : (ht + 1) * 128], ident)

                    def evac():
                        evac_seq[0] ^= 1
                        if evac_seq[0]:
                            nc.scalar.copy(xt_v[:, :, c0 : c0 + 2, :], pt)
                        else:
                            nc.vector.tensor_copy(xt_v[:, :, c0 : c0 + 2, :], pt)
                    return evac

                # warm the PE clock (HAM) while the first gathers land:
                # harmless matmuls on the identity into a scratch bank.
                warm = tpp.tile([128, 512], f32, tag="pt")
                for i in range(24):
                    nc.tensor.matmul(
                        warm[:, :128], ident, ident,
                        start=(i == 0), stop=(i == 23))

                for c0 in (20, 22):
                    transpose_pair(c0)()

                # --- evac engine alternation (2:1 toward ACT; DVE is the
                # hotter engine during the era) --------------------------
                eng_flip = [0]

                def _evac_on_act():
                    eng_flip[0] = (eng_flip[0] + 1) % 3
                    return eng_flip[0] != 0

                def evac_relu(dst, ps, bias_ap):
                    if _evac_on_act():
                        nc.scalar.activation(dst, ps, AF.Relu, bias=bias_ap)
                    else:
                        nc.vector.tensor_scalar(
                            out=dst, in0=ps, scalar1=bias_ap, scalar2=0.0,
                            op0=ALU.add, op1=ALU.max)

                def evac_ident(dst, ps, bias_ap):
                    if _evac_on_act():
                        nc.scalar.activation(dst, ps, AF.Identity, bias=bias_ap)
                    else:
                        nc.vector.tensor_scalar(
                            out=dst, in0=ps, scalar1=bias_ap, scalar2=None,
                            op0=ALU.add)

                # --- units ---------------------------------------------
                def b_unit(mat, ch, mt):
                    w_s, b_s = {
                        "z": (wzt_s, bz_s), "h": (wht_s, bh_s),
                        "r": (wr_s, br_s)}[mat]
                    dst = {"z": azr[:, 0], "h": ah, "r": azr[:, 1]}[mat]
                    msl = slice(mt * 128, (mt + 1) * 128)
                    csl = slice(ch * 512, (ch + 1) * 512)
                    ps = hps_p.tile([128, 512], f32, tag="hps")
                    for kt in range(2):
                        nc.tensor.matmul(
                            ps, w_s[:, kt, msl], xt[:, kt, csl],
                            start=(kt == 0), stop=(kt == 1))
                    return lambda: evac_ident(dst[:, mt, csl], ps, b_s[:, mt:mt+1])

                def p0_unit(ch, mt):
                    msl = slice(mt * 128, (mt + 1) * 128)
                    csl = slice(ch * 512, (ch + 1) * 512)
                    ps = hps_p.tile([128, 512], f32, tag="hps")
                    for kt in range(2):
                        nc.tensor.matmul(
                            ps, uwx_s[:, kt, msl], xt[:, kt, csl],
                            start=(kt == 0), stop=False)
                    for kt in range(2):
                        nc.tensor.matmul(
                            ps, uwh_s[:, kt, msl], mfq_cols(kt, 8 * ch, 8),
                            start=False, stop=False)
                    nc.tensor.matmul(
                        ps, uwl_s[:, msl], tvrep[:, :8, :],
                        start=False, stop=True)
                    return lambda: evac_relu(p0a[:, mt, csl], ps, ub_s[:, mt:mt+1])

                def p1_unit(u, mt):
                    n0 = 4 * u
                    nn = min(4, NF - n0)
                    cw = nn * BC
                    msl = slice(mt * 128, (mt + 1) * 128)
                    csl = slice(n0 * BC, n0 * BC + cw)
                    ps = hps_p.tile([128, 512], f32, tag="hps")
                    psv = ps[:, :cw]
                    for kt in range(2):
                        nc.tensor.matmul(
                            psv, uwx_s[:, kt, msl], xt[:, kt, csl],
                            start=(kt == 0), stop=False)
                    for kt in range(2):
                        nc.tensor.matmul(
                            psv, uwh_s[:, kt, msl], hbw_f[:, kt, csl],
                            start=False, stop=False)
                    nc.tensor.matmul(
                        psv, uwl_s[:, msl], tvrep[:, :nn, :],
                        start=False, stop=True)
                    return lambda: evac_relu(p1a[:, mt, csl], psv, ub_s[:, mt:mt+1])

                def q1_unit(ch, mt):
                    msl = slice(mt * 128, (mt + 1) * 128)
                    csl = slice(ch * 512, (ch + 1) * 512)
                    ps = hps_p.tile([128, 512], f32, tag="hps")
                    for kt in range(2):
                        nc.tensor.matmul(
                            ps, wwh_s[:, kt, msl], mfq_cols(kt, 8 * ch, 8),
                            start=(kt == 0), stop=False)
                    nc.tensor.matmul(
                        ps, wwl_s[:, msl], tvrep[:, :8, :],
                        start=False, stop=True)
                    return lambda: evac_relu(q1a[:, mt, csl], ps, wb_s[:, mt:mt+1])

                def q2_unit(j, half):
                    # vocab half of one logits row-tile through the shared
                    # head-pipe pool; softmax partials (sume/qt/rmax) are
                    # accumulated per half and combined in three tail ops.
                    n0, nn = (0, 512) if half == 0 else (512, V - 512)
                    col = half * 24 + j
                    psq = hps_p.tile([128, 512], f32, tag="hps")
                    pv = psq[:, :nn]
                    jsl = slice(j * 128, (j + 1) * 128)
                    for kt in range(2):
                        nc.tensor.matmul(
                            pv, q1a[:, kt, jsl], wo_s[:, kt, n0 : n0 + nn],
                            start=(kt == 0), stop=(kt == 1))
                    if wob_nonzero:
                        wv = wob_s[:]
                        wb_b = bass.AP(
                            tensor=wv.tensor, offset=wv.offset + n0,
                            ap=[[0, 128], [1, nn]])
                        nc.vector.tensor_add(pv, pv, wb_b)

                    def softmax():
                        nc.scalar.activation(
                            scr[:, :nn], pv, AF.Exp,
                            accum_out=sume_acc[:, col : col + 1])
                        nc.vector.scalar_tensor_tensor(
                            out=scr2[:, :nn], in0=iota_f[:, n0 : n0 + nn],
                            scalar=qtgt_s[:, j : j + 1],
                            in1=pv, op0=ALU.is_equal, op1=ALU.mult,
                            accum_out=qt_acc[:, col : col + 1])
                        nc.vector.reduce_max(
                            rmax_acc[:, col : col + 1], pv, axis=AX.X)
                    return softmax

                # p2 sub-units: a few p1-row tiles reduced against Us as
                # soon as their relu arena columns are complete, straight
                # into the matching p_sb slice (no persistent psum).
                p_sb = sp.tile([128, 48], f32, tag="psb")

                def p2_unit(src, j0, nj, col0):
                    ps = hps_p.tile([128, 512], f32, tag="hps")
                    for j in range(j0, j0 + nj):
                        for kt in range(2):
                            nc.tensor.matmul(
                                ps[:, j - j0 : j - j0 + 1],
                                src[:, kt, j * 128 : (j + 1) * 128],
                                us_s[:, kt, :],
                                start=(kt == 0), stop=(kt == 1))
                    return lambda: nc.scalar.activation(
                        p_sb[:, col0 : col0 + nj], ps[:, :nj],
                        AF.Identity, bias=usb_s[:, 0:1])

                # --- pump scheduler ------------------------------------
                pending = []
                seq_ctr = [0]

                def enq(ready, kind, fn, deadline=10**9):
                    pending.append([ready, seq_ctr[0], kind, deadline, fn])
                    seq_ctr[0] += 1

                evacs_next = []
                q1_emitted = {}
                p0_emitted = {}
                p1_emitted = {}
                cur_t = [0]

                def flush_evacs():
                    for ev in evacs_next:
                        ev()
                    evacs_next.clear()

                def pump(t, nh=3):
                    flush_evacs()
                    budget = {"hps": nh, "tps": 1}
                    pending.sort(key=lambda u: (u[0], u[1]))
                    for u in list(pending):
                        ready, _, kind, deadline, fn = u
                        if ready > t or budget[kind] == 0:
                            continue
                        assert t <= deadline, f"unit past deadline at iter {t}"
                        budget[kind] -= 1
                        pending.remove(u)
                        evacs_next.append(fn())

                # phase-B chunks 5,0 then 4,1 in the prefix (the scan
                # reads them by iters 0 and 8); software-pipelined evacs
                # (hps bufs=2 -> <=2 in flight).
                def b_prefix(chunks):
                    prev_ev = None
                    for ch in chunks:
                        for mat in ("z", "r", "h"):
                            for mt in range(2):
                                ev = b_unit(mat, ch, mt)
                                if prev_ev is not None:
                                    prev_ev()
                                prev_ev = ev
                    prev_ev()

                # all of chunk 5 (needed by bwd iter 0) flows before
                # chunk 0's transposes even start.
                # (transpose pairs 20,22 were emitted above; 0,2 follow.)
                b_prefix((5,))
                for c0 in (0, 2):
                    transpose_pair(c0)()
                b_prefix((0,))
                # all remaining transposes and B chunks are era units so
                # the scan starts right after B(5,0): pair at pump p
                # writes xt at pump p+1 (evacs run first); B chunk ready
                # gates on its transposes' evacs, deadline = first-read
                # iteration minus 2 (unit at pump p -> azr written at
                # pump p+1, before scan_iter(p+2)'s prefill).
                for c0, p in ((16, 0), (18, 1), (4, 2), (6, 3),
                              (12, 4), (14, 5), (8, 6), (10, 7)):
                    enq(p, "tps", lambda cc=c0: transpose_pair(cc), deadline=p)
                for ch, rdy, dl in ((4, 2, 8), (1, 4, 8), (3, 6, 16), (2, 8, 16)):
                    for mat in ("z", "r", "h"):
                        for mt in range(2):
                            enq(rdy, "hps",
                                lambda m=mat, c=ch, k=mt: b_unit(m, c, k),
                                deadline=dl - 2)

                def _after_p0(c, k):
                    ev = p0_unit(c, k)
                    def done():
                        ev()
                        p0_emitted[c] = p0_emitted.get(c, 0) + 1
                        if p0_emitted[c] == 2:
                            # p0a chunk c complete -> p-row tiles 4c..4c+3
                            enq(cur_t[0] + 1, "hps",
                                lambda cc=c: p2_unit(p0a, 4 * cc, 4, 4 * cc))
                    return done

                def _after_p1(u, k):
                    ev = p1_unit(u, k)
                    def done():
                        ev()
                        p1_emitted[u] = p1_emitted.get(u, 0) + 1
                        if p1_emitted[u] == 2:
                            # p1a nodes 4u..4u+3 complete -> tiles 2u,2u+1
                            enq(cur_t[0] + 1, "hps",
                                lambda uu=u: p2_unit(p1a, 2 * uu, 2, 24 + 2 * uu))
                    return done

                for ch in range(6):
                    rdy = min(8 * ch + 7, NF)
                    for mt in range(2):
                        enq(rdy, "hps", lambda c=ch, k=mt: _after_p0(c, k))

                        def _q1(c=ch, k=mt):
                            ev = q1_unit(c, k)
                            q1_emitted[c] = q1_emitted.get(c, 0) + 1
                            if q1_emitted[c] == 2:
                                for j in range(4 * c, 4 * c + 4):
                                    for hf in range(2):
                                        enq(cur_t[0] + 1, "hps",
                                            lambda jj=j, h=hf: q2_unit(jj, h))
                            return ev
                        enq(rdy, "hps", _q1)
                for u in range(12):
                    n0 = 4 * u
                    nn = min(4, NF - n0)
                    rdy = max(max(NF - 1 - n, n - 1) for n in range(n0, n0 + nn)) + 1
                    for mt in range(2):
                        enq(rdy, "hps", lambda uu=u, k=mt: _after_p1(uu, k))

                # --- scan state tiles (fused across chains) ------------
                zr_sb = pp.tile([128, 2, 2, 2, BC], wdt, tag="zrsb")  # [gate,mt,ch,b]
                rm_fb = pp.tile([128, 2, 2, BC], wdt, tag="rmfb")

                def zr_prefill(t):
                    # A_z/A_r pulled into the zr bank; independent of the
                    # scan state, so it is emitted at the END of iteration
                    # t-1 (before the pumped head units) and runs in the
                    # me-combine window on the other zr buffer.
                    s_f, s_b = t, NF - t
                    ps = zr_p.tile([128, 2, 2, 2, BC], f32, tag="zr")
                    nc.tensor.matmul(
                        ps[:, :, :, 0, :], ident,
                        azr[:, :, :, s_f * BC : (s_f + 1) * BC],
                        start=True, stop=False)
                    nc.tensor.matmul(
                        ps[:, :, :, 1, :], ident,
                        azr[:, :, :, s_b * BC : (s_b + 1) * BC],
                        start=False, stop=False)
                    return ps

                def scan_iter(t, ps):
                    """Interleaved fwd+bwd GRU iteration t (fused chains).

                    src_f = t, src_b = 47-t; dst(t-1) == src(t) on a chain.
                    psum zr: [gate2, mt2, ch2, 64]; psum h: [mt2, ch2, 64].
                    Critical path: me(t-1) -> r mms -> tanh_r -> rm' ->
                    h rm-mms -> tanh_h -> zpmt -> me(t); everything else
                    (z gate, h me-part mms, ozs) runs off that path.
                    rm = (1+r')/2 * me is folded into the h matmuls:
                    WhB^T rm = WhB2^T(r' me) + WhB2^T me  (WhB2 = WhB/2,
                    prescaled host-side).
                    """
                    s_f, s_b = t, NF - t
                    s_ap = hist[:, :, t, :, :]
                    # r-gate state matmuls first: tanh_r is on the critical
                    # path, z is not.
                    gates = [(1, ur_s), (0, wzb_s)] if t > 0 else [(0, wzb_s)]
                    for gi, (g, w_s) in enumerate(gates):
                        for mt in range(2):
                            msl = slice(mt * 128, (mt + 1) * 128)
                            for kt in range(2):
                                nc.tensor.matmul(
                                    ps[:, g, mt, :, :], w_s[:, kt, msl],
                                    s_ap[:, kt, :, :],
                                    start=False,
                                    stop=(gi == len(gates) - 1 and mt == 1
                                          and kt == 1))
                    if t > 0:
                        nc.scalar.activation(zr_sb[:, 1], ps[:, 1], AF.Tanh)
                    # H bank: A_h prefill + the state me-part (off-path)
                    ps_h = h_p.tile([128, 2, 2, BC], f32, tag="hh")
                    nc.tensor.matmul(
                        ps_h[:, :, 0, :], ident,
                        ah[:, :, s_f * BC : (s_f + 1) * BC],
                        start=True, stop=False)
                    nc.tensor.matmul(
                        ps_h[:, :, 1, :], ident,
                        ah[:, :, s_b * BC : (s_b + 1) * BC],
                        start=False, stop=(t == 0))
                    if t > 0:
                        for mt in range(2):
                            msl = slice(mt * 128, (mt + 1) * 128)
                            for kt in range(2):
                                nc.tensor.matmul(
                                    ps_h[:, mt, :, :], whb_s[:, kt, msl],
                                    s_ap[:, kt, :, :],
                                    start=False, stop=False)
                    nc.scalar.activation(zr_sb[:, 0], ps[:, 0], AF.Tanh)
                    if t > 0:
                        # rm' = r' * me(t-1); h rm-part matmuls (on path);
                        # WhB^T rm = WhB2^T (r' me) + WhB2^T me, WhB2
                        # prescaled on the host.
                        nc.vector.tensor_mul(rm_fb, zr_sb[:, 1], s_ap)
                        for mt in range(2):
                            msl = slice(mt * 128, (mt + 1) * 128)
                            for kt in range(2):
                                nc.tensor.matmul(
                                    ps_h[:, mt, :, :], whb_s[:, kt, msl],
                                    rm_fb[:, kt, :, :],
                                    start=False, stop=(mt == 1 and kt == 1))

                    # off-path: zp = 0.5 z' + 0.5 ; ozs = (0.5 - 0.5 z') s
                    zp = st.tile([128, 2, 2, BC], wdt, tag="zp")
                    nc.vector.tensor_scalar(
                        out=zp, in0=zr_sb[:, 0], scalar1=0.5, scalar2=0.5,
                        op0=ALU.mult, op1=ALU.add)
                    on = st.tile([128, 2, 2, BC], wdt, tag="on")
                    nc.vector.tensor_scalar(
                        out=on, in0=zr_sb[:, 0], scalar1=-0.5, scalar2=0.5,
                        op0=ALU.mult, op1=ALU.add)
                    ozs = st.tile([128, 2, 2, BC], wdt, tag="ozs")
                    nc.vector.tensor_mul(ozs, on, s_ap)
                    # on path: mt = tanh(psum_h); me = zp*mt + ozs
                    mt_sb = st.tile([128, 2, 2, BC], wdt, tag="mts")
                    nc.scalar.activation(mt_sb, ps_h, AF.Tanh)
                    zpmt = st.tile([128, 2, 2, BC], wdt, tag="zpmt")
                    nc.vector.tensor_mul(zpmt, zp, mt_sb)
                    nc.vector.tensor_add(hist[:, :, t + 1, :, :], zpmt, ozs)
                    # h_v adds ready at t: hbw[n] = m_b[slot 47-n] + m_f[slot n]
                    for n in range(NF):
                        if max(NF - 1 - n, n - 1) == t:
                            nc.vector.tensor_add(
                                hbw[:, :, n, :],
                                hist[:, :, QBLK - 1 - n, 1, :],
                                hist[:, :, n, 0, :])

                for t in range(NF):
                    cur_t[0] = t
                    scan_iter(t, zr_prefill(t))
                    pump(t, nh=3)
                t = NF
                while pending or evacs_next:
                    cur_t[0] = t
                    pump(t, nh=3)
                    t += 1

                # --- tail: combine vocab-half softmax partials, BCE ----
                sume_c = sp.tile([128, 24], f32, tag="sumc")
                nc.vector.tensor_add(sume_c, sume_acc[:, :24], sume_acc[:, 24:])
                qt_c = sp.tile([128, 24], f32, tag="qtc")
                nc.vector.tensor_add(qt_c, qt_acc[:, :24], qt_acc[:, 24:])
                rmax_c = sp.tile([128, 24], f32, tag="rmc")
                nc.vector.tensor_tensor(
                    out=rmax_c, in0=rmax_acc[:, :24], in1=rmax_acc[:, 24:],
                    op=ALU.max)
                qc_t = sp.tile([128, 24], f32, tag="qct")
                nc.vector.tensor_tensor(
                    out=qc_t, in0=qt_c, in1=rmax_c, op=ALU.is_ge)
                ab_t = sp.tile([128, 48], f32, tag="abt")
                nc.scalar.activation(ab_t, p_sb, AF.Abs)
                en_t = sp.tile([128, 48], f32, tag="ent")
                nc.scalar.activation(en_t, ab_t, AF.Exp, scale=-1.0)
                rl_t = sp.tile([128, 48], f32, tag="rlt")
                nc.scalar.activation(rl_t, p_sb, AF.Relu)
                l1p_t = sp.tile([128, 48], f32, tag="l1p")
                nc.scalar.activation(l1p_t, en_t, AF.Ln, bias=1.0)
                lse_t = sp.tile([128, 24], f32, tag="lse")
                nc.scalar.activation(lse_t, sume_c, AF.Ln)

                sp_t = sp.tile([128, 48], f32, tag="spt")
                nc.vector.tensor_add(sp_t, l1p_t, rl_t)
                ptt = sp.tile([128, 48], f32, tag="ptt")
                nc.vector.tensor_mul(ptt, p_sb, ptgt_s)
                bce = sp.tile([128, 48], f32, tag="bce")
                nc.vector.tensor_sub(bce, sp_t, ptt)
                nc.vector.reduce_sum(outp_s[:, 0:1], bce, axis=AX.X)
                gtz = sp.tile([128, 48], f32, tag="gtz")
                nc.vector.tensor_scalar(
                    out=gtz, in0=p_sb, scalar1=0.0, scalar2=None, op0=ALU.is_gt)
                pcr = sp.tile([128, 48], f32, tag="pcr")
                nc.vector.tensor_tensor(out=pcr, in0=gtz, in1=ptgt_s,
                                        op=ALU.is_equal)
                nc.vector.reduce_sum(outp_s[:, 1:2], pcr, axis=AX.X)
                nc.vector.reduce_sum(outp_s[:, 2:3], lse_t, axis=AX.X)
                nc.vector.reduce_sum(outp_s[:, 3:4], qt_c, axis=AX.X)
                nc.vector.reduce_sum(outp_s[:, 4:5], qc_t, axis=AX.X)
            nc.sync.dma_start(out=outp[:], in_=outp_s)

    nc.finalize()
    return nc


def _get_nc(wob_nonzero: bool):
    key = ("nc", wob_nonzero, BF16)
    if key not in _CACHE:
        _CACHE[key] = _build(wob_nonzero)
    return _CACHE[key]


def _wdt_np():
    if BF16:
        import ml_dtypes

        return ml_dtypes.bfloat16
    return np.float32


def _prep_inputs(inputs):
    f = lambda k: np.ascontiguousarray(np.asarray(inputs[k]), dtype=np.float32)
    wdt = _wdt_np()
    w = lambda a: np.ascontiguousarray(a).astype(wdt)
    wid = np.asarray(inputs["wid"]).astype(np.int64).reshape(B, N)
    tree_vec = f("tree_vec")
    Wz, bz = f("Wz"), f("bz")
    Wr_, Ur_, br = f("Wr"), f("Ur"), f("br")
    Wh, bh = f("Wh"), f("bh")
    W_w, W_b = f("W_w"), f("W_b")
    U_w, U_b = f("U_w"), f("U_b")
    Wo_w, Wo_b = f("Wo_w"), f("Wo_b")
    Us_w, Us_b = f("Us_w"), f("Us_b")
    emb = f("embedding")

    def c2(v):
        return np.ascontiguousarray(v.reshape(2, 128).T)

    # sigma(x) = (1+tanh(x/2))/2: z and r pre-activations halved host-side.
    # WhB is halved too: WhB^T rm = (WhB/2)^T (r' me) + (WhB/2)^T me.
    shared = dict(
        emb=w(emb),
        WzT=w(0.5 * Wz[:H]), WzB=w(0.5 * Wz[H:]),
        WhT=w(Wh[:H]), WhB=w(0.5 * Wh[H:]),
        Wr=w(0.5 * Wr_), Ur=w(0.5 * Ur_),
        UwX=w(U_w[:H]), UwH=w(U_w[H : 2 * H]), UwL=w(U_w[2 * H :]),
        WwH=w(W_w[:H]), WwL=w(W_w[H:]),
        Wo=w(Wo_w), Us=w(Us_w),
        bz2=c2(0.5 * bz), bh2=c2(bh), br2=c2(0.5 * br),
        ub2=c2(U_b), wb2=c2(W_b),
        usb=np.full((128, 1), float(Us_b.reshape(-1)[0]), np.float32),
    )
    wob_nonzero = bool(np.any(Wo_b != 0))
    if wob_nonzero:
        shared["wob"] = Wo_b.reshape(1, V)

    ii, pprt = np.meshgrid(np.arange(48), np.arange(128), indexing="xy")
    tblk = 2 * ii + pprt // 64
    ptgt = np.ascontiguousarray((tblk <= 46).astype(np.float32))

    in_maps = []
    for c in range(NC):
        w2 = wid[c * BC : (c + 1) * BC]
        flat = np.ascontiguousarray(w2.T).reshape(-1)
        m = dict(shared)
        m["gidx"] = np.ascontiguousarray(flat.reshape(24, 128)).astype(np.int32)
        m["tvt"] = np.ascontiguousarray(
            np.tile(tree_vec[c * BC : (c + 1) * BC].T, (1, 8))
        ).astype(wdt)
        m["qtgt"] = np.ascontiguousarray(flat.reshape(24, 128).T).astype(np.float32)
        m["ptgt"] = ptgt
        in_maps.append(m)
    return in_maps, wob_nonzero, float(Us_b.reshape(-1)[0])


def _combine(results, us_b):
    S = np.zeros(8, np.float64)
    for r in results:
        S += np.asarray(r["outp"], np.float64).sum(axis=0)
    pad_bce = max(us_b, 0.0) + np.log1p(np.exp(-abs(us_b)))
    pad_corr = 1.0 if us_b <= 0 else 0.0
    n_pad = NC * (PPAD - PROWS)
    p_loss = (S[0] - n_pad * pad_bce) / B
    p_acc = (S[1] - n_pad * pad_corr) / (PBLK * B)
    q_loss = (S[2] - S[3]) / B
    q_acc = S[4] / (QBLK * B)
    return np.array([q_loss, p_loss, q_acc, p_acc], np.float32)


def kernel(**inputs) -> np.ndarray:
    from concourse.bass_utils import run_bass_kernel_spmd

    in_maps, wob_nonzero, us_b = _prep_inputs(inputs)
    nc = _get_nc(wob_nonzero)
    res = run_bass_kernel_spmd(nc, in_maps, list(range(NC)))
    return _combine(res.results, us_b)
